# revision 56
# baseline (speedup 1.0000x reference)
"""Multi-head causal attention (B=4, S=2048, D=1024, H=16, Hd=64) on 8 trn2 cores.

Sharding: data-parallel over batch (4) x tensor-parallel over heads (2 groups
of 8 heads). Core c handles batch c//2 and heads 8*(c%2)..8*(c%2)+7:
  - wq/wk/wv column-parallel (each core owns 512 of the 1024 output dims),
  - wo row-parallel (partial outputs summed on host).

Device-side per core:
  phase 1: qT/kT (transposed, [dq,S]) and v (natural, [S,hd]) projections
  phase 2: per head-pair d, q-swath j: scoresT = kT.T-chunk @ qT-swath (row-
           tiled pair of K=64 matmuls), causal additive mask on diagonal
           tiles, exp on ACT (no max subtraction: scores are O(1), exp is
           safe), PV matmul with a ones-column appended to v so the softmax
           denominator falls out of the same matmul, then normalize.
  phase 3: out_partial = attnT.T @ woT  (row-parallel wo)

Host side: shard/transposes, pair-sum of partials, + wo@bv + bo correction
(bk provably cancels in softmax; bv commutes to a constant because softmax
rows sum to 1).

Math note: softmax computed without max-subtraction (scores ~ N(0,1), exp
overflow impossible in fp32); masked entries get -1e30 pre-exp -> exp = 0.
"""
import sys

sys.path.insert(0, "/opt/trn_rl_repo")

import numpy as np

from concourse import bacc, mybir, tile
from concourse.bass_utils import run_bass_kernel_spmd

B, S, D = 4, 2048, 1024
H, HD = 16, 64
HPC = 8        # heads per core
DPC = HPC * HD  # 512 projection dims per core
SW = 512       # q swath width
NSW = S // SW  # 4
NT = S // 128  # 16 token tiles
ND = D // 128  # 8 contraction chunks

# matmul dtype mode: "f32" (exact, 4x slow), "f32r" (full speed, ~tf32ish),
# "bf16" (full speed, least precise, half DMA/SBUF)
MODE = "bf16"

F32 = mybir.dt.float32
EXPF = mybir.ActivationFunctionType.Exp

_NC_CACHE = {}


def _mm_dt(mode):
    import ml_dtypes
    if mode == "bf16":
        return mybir.dt.bfloat16, ml_dtypes.bfloat16
    if mode in ("f32r", "f32r_hi"):
        # float32r: fp32 storage, PE reads reduced mantissa at full rate.
        # np-side arrays stay fp32.
        return mybir.dt.float32r, np.float32
    return F32, np.float32


def _build(mode):
    mdt, _ = _mm_dt(mode)
    # PV-stage dtype: bf16 operands (exp weights + v) halve SBUF at a
    # ~2e-3 rel-err cost; f32r_hi keeps them f32r (~3.5e-4) with tighter
    # buffer counts to fit SBUF.
    pdt = mybir.dt.bfloat16 if mode == "f32r" else mdt
    hi = mode != "f32r"

    def mc(ap):
        return ap

    nc = bacc.Bacc("TRN2", target_bir_lowering=False, debug=False, num_devices=8)

    xT_d = nc.dram_tensor("xT", [D, S], mdt, kind="ExternalInput").ap()
    wqT_d = nc.dram_tensor("wqT", [D, DPC], mdt, kind="ExternalInput").ap()
    wkT_d = nc.dram_tensor("wkT", [D, DPC], mdt, kind="ExternalInput").ap()
    wvT_d = nc.dram_tensor("wvT", [D, DPC], mdt, kind="ExternalInput").ap()
    woT_d = nc.dram_tensor("woT", [DPC, D], mdt, kind="ExternalInput").ap()
    bqT_d = nc.dram_tensor("bqT", [128, 4], F32, kind="ExternalInput").ap()
    cmT_d = nc.dram_tensor("cmT", [128, 128], mdt, kind="ExternalInput").ap()
    id_d = nc.dram_tensor("id128", [128, 128], mdt, kind="ExternalInput").ap()
    out_d = nc.dram_tensor("out", [S, D], F32, kind="ExternalOutput").ap()

    xT_r = xT_d.rearrange("(c p) s -> p c s", p=128)
    wqT_r = wqT_d.rearrange("(c p) n -> p c n", p=128)
    wkT_r = wkT_d.rearrange("(c p) n -> p c n", p=128)
    wvT_r = wvT_d.rearrange("(c p) n -> p c n", p=128)
    woT_r = woT_d.rearrange("(c p) n -> p c n", p=128)

    with tile.TileContext(nc) as tc:
        with (
            tc.tile_pool(name="pers", bufs=1) as pp,
            tc.tile_pool(name="qts", bufs=2) as qp,
            tc.tile_pool(name="aots", bufs=3) as aop,
            tc.tile_pool(name="xp", bufs=1 if hi else 3) as xp,
            tc.tile_pool(name="exp", bufs=3 if hi else 6) as ep,
            tc.tile_pool(name="rp", bufs=3) as rp,
            tc.tile_pool(name="stp", bufs=4) as sp3,
            tc.tile_pool(name="scp", bufs=3, space="PSUM") as ps2,
            tc.tile_pool(name="pvp", bufs=1, space="PSUM") as pvp,
        ):
            kT = [pp.tile([128, S], mdt, tag=f"kT{d}", name=f"kT{d}") for d in range(4)]
            v3 = [pp.tile([128, HPC, HD + 1], pdt, tag=f"v{t}", name=f"v{t}") for t in range(NT)]
            wqt = pp.tile([128, ND, DPC], mdt, tag="wqt", name="wqt")
            wkt = pp.tile([128, ND, DPC], mdt, tag="wkt", name="wkt")
            wvt = pp.tile([128, ND, DPC], mdt, tag="wvt", name="wvt")
            wot = pp.tile([128, 4, D], mdt, tag="wot", name="wot")
            bqT = pp.tile([128, 4], F32, tag="bqT", name="bqT")
            zb = pp.tile([128, 1], F32, tag="zb", name="zb")
            cmT = pp.tile([128, 128], mdt, tag="cmT", name="cmT")
            id128 = pp.tile([128, 128], mdt, tag="id128", name="id128")
            # wqt first: the opening projections only need it (+ xsw(0));
            # chunked so the first proj steps start after ~1/4 of the bytes.
            # bqT/cmT/id128 are only consumed from the first drain/attention
            # on, so they queue behind the critical path.
            nc.sync.dma_start(wqt[:, 0:2], wqT_r[:, 0:2])
            nc.vector.memset(zb[:], 0.0)
            for t in range(NT):
                # ones column for the in-PV softmax denominator; v3 tiles are
                # persistent so this is set once, drains only write [0:HD]
                nc.vector.memset(v3[t][:, :, HD:HD + 1], 1.0)

            qtab = {sj: [None] * 4 for sj in range(NSW)}  # per (sj, dd) qT tile
            aocur = [None] * 4   # per-dd current swath attnT tile

            # persistent per-swath x tiles: unique tags mean no pool-rotation
            # WAR gates, so every x DMA can run during the prologue
            xs = [pp.tile([128, ND, SW], mdt, tag=f"xsw{j}", name=f"xsw{j}")
                  for j in range(NSW)]

            filler = []  # FIFO of emission closures (each ~2 matmuls of filler)

            def proj_pair_qk(sj, xsw, which, da, db):
                # two projection outputs share one 2-bank psum tile; dk-steps
                # are queued as filler closures woven into attention i-loops
                wt = wqt if which == "q" else wkt
                box = {}

                def step(dk, box=box):
                    if dk == 0:
                        box["ps"] = ps2.tile([128, 2 * SW], F32, tag="sc", name=f"p{which}{sj}_{da}")
                    ps = box["ps"]
                    nc.tensor.matmul(
                        ps[:, 0:SW], mc(wt[:, dk, 128 * da:128 * da + 128]),
                        mc(xsw[:, dk, :]), start=(dk == 0), stop=(dk == ND - 1))
                    nc.tensor.matmul(
                        ps[:, SW:2 * SW], mc(wt[:, dk, 128 * db:128 * db + 128]),
                        mc(xsw[:, dk, :]), start=(dk == 0), stop=(dk == ND - 1))

                def drain(box=box):
                    ps = box["ps"]
                    cols = slice(SW * sj, SW * (sj + 1))
                    if which == "q":
                        for half, dd in ((0, da), (1, db)):
                            qt = qp.tile([128, SW], mdt, tag=f"qT{dd}", name=f"qT{dd}_{sj}")
                            nc.vector.tensor_scalar_add(
                                qt[:], ps[:, SW * half:SW * (half + 1)], bqT[:, dd:dd + 1])
                            qtab[sj][dd] = qt
                    else:
                        for half, dd in ((0, da), (1, db)):
                            nc.vector.tensor_copy(kT[dd][:, cols], ps[:, SW * half:SW * (half + 1)])

                for dk in range(ND):
                    filler.append(lambda dk=dk: step(dk))
                filler.append(drain)

            def proj_pair_v(sj, xsw, ta, tb):
                box = {}

                def step(dk, box=box):
                    if dk == 0:
                        box["ps"] = ps2.tile([128, 2 * SW], F32, tag="sc", name=f"pv{sj}_{ta}")
                    ps = box["ps"]
                    nc.tensor.matmul(
                        ps[:, 0:SW], mc(xsw[:, dk, 128 * ta:128 * ta + 128]),
                        mc(wvt[:, dk, :]), start=(dk == 0), stop=(dk == ND - 1))
                    nc.tensor.matmul(
                        ps[:, SW:2 * SW], mc(xsw[:, dk, 128 * tb:128 * tb + 128]),
                        mc(wvt[:, dk, :]), start=(dk == 0), stop=(dk == ND - 1))

                def drain(box=box):
                    ps = box["ps"]
                    for half, tloc in ((0, ta), (1, tb)):
                        t = 4 * sj + tloc
                        nc.vector.tensor_copy(
                            v3[t][:, :, 0:HD],
                            ps[:, SW * half:SW * (half + 1)].rearrange("p (h e) -> p h e", h=HPC))

                for dk in range(ND):
                    filler.append(lambda dk=dk: step(dk))
                filler.append(drain)

            def pop_filler(n):
                for _ in range(n):
                    if not filler:
                        return
                    filler.pop(0)()

            def emit_scores(dd, sj, i, qt):
                krows = slice(128 * i, 128 * (i + 1))
                # diagonal key tiles: queries below c0 are fully masked, skip
                # their score columns
                c0 = 128 * (i - 4 * sj) if i >= 4 * sj else 0
                ps = ps2.tile([128, 2 * SW], F32, tag="sc", name=f"sc{dd}_{sj}_{i}")
                if c0 == 0 and i < 4 * sj:
                    # off-diagonal tile: plain full-width scores
                    nc.tensor.matmul(ps[:, 0:SW], mc(kT[dd][0:64, krows]),
                                     mc(qt[0:64, :]))
                    nc.tensor.matmul(ps[:, SW:2 * SW], mc(kT[dd][64:128, krows]),
                                     mc(qt[64:128, :]))
                    return ps
                # diagonal tile: preload the additive causal mask into the
                # 128-wide diagonal block via a PE matmul (cmT @ I), then
                # accumulate scores on top; exp can then read psum directly
                # with no vector mask-add in the chain.
                c1 = c0 + 128
                for g in range(2):
                    nc.tensor.matmul(ps[:, g * SW + c0:g * SW + c1], mc(cmT[:]),
                                     mc(id128[:]), start=True, stop=False)
                nc.tensor.matmul(ps[:, c0:c1], mc(kT[dd][0:64, krows]),
                                 mc(qt[0:64, c0:c1]), start=False, stop=True)
                nc.tensor.matmul(ps[:, SW + c0:SW + c1], mc(kT[dd][64:128, krows]),
                                 mc(qt[64:128, c0:c1]), start=False, stop=True)
                if c1 < SW:
                    nc.tensor.matmul(ps[:, c1:SW], mc(kT[dd][0:64, krows]),
                                     mc(qt[0:64, c1:SW]))
                    nc.tensor.matmul(ps[:, SW + c1:2 * SW], mc(kT[dd][64:128, krows]),
                                     mc(qt[64:128, c1:SW]))
                return ps

            def emit_tail(dd, sj, i, ps, pv0, pv1, last):
                h0, h1 = 2 * dd, 2 * dd + 1
                t = i - 4 * sj
                c0 = 128 * t if t >= 0 else 0
                ex = ep.tile([128, 2 * SW], pdt, tag="ex", name=f"ex{dd}_{sj}_{i}")
                if t >= 0:
                    pse = ps[:].rearrange("p (g q) -> p g q", g=2)[:, :, c0:SW]
                    exe = ex[:].rearrange("p (g q) -> p g q", g=2)[:, :, c0:SW]
                    nc.scalar.activation(exe, pse, EXPF, bias=zb[:], scale=0.125)
                else:
                    nc.scalar.activation(ex[:], ps[:], EXPF, bias=zb[:], scale=0.125)
                nc.tensor.matmul(
                    pv0[0:HD + 1, c0:SW], mc(v3[i][:, h0, :]), mc(ex[:, c0:SW]),
                    start=(i == 0), stop=(i == last))
                nc.tensor.matmul(
                    pv1[0:HD + 1, c0:SW], mc(v3[i][:, h1, :]), mc(ex[:, SW + c0:2 * SW]),
                    start=(i == 0), stop=(i == last))

            def emit_norm(dd, sj, pv, hh):
                rb_ = rp.tile([64, SW], F32, tag=f"rb{hh}", name=f"rb{hh}_{dd}_{sj}")
                r_ = rp.tile([1, SW], F32, tag=f"r{hh}", name=f"r{hh}_{dd}_{sj}")
                nc.vector.tensor_copy(rb_[0:1, :], pv[HD:HD + 1, :])
                nc.vector.reciprocal_approx_fast(out=r_[0:1, :], in_=rb_[0:1, :])
                nc.gpsimd.partition_broadcast(rb_[0:64, :], r_[0:1, :])
                if hh == 0:
                    nc.vector.tensor_mul(aocur[dd][0:64, :], pv[0:64, :], rb_[0:64, :])
                else:
                    # DVE cross-quadrant write: shift h1's normalized output up
                    # to partitions 64:128 without a DMA
                    nc.vector.tensor_mul(aocur[dd][64:128, :], pv[0:64, :], rb_[0:64, :])

            def emit_att(dd, sj, qt):
                last = 4 * sj + 3
                pv0 = pvp.tile([128, SW], F32, tag="pv0", name=f"pvh0_{dd}_{sj}")
                pv1 = pvp.tile([128, SW], F32, tag="pv1", name=f"pvh1_{dd}_{sj}")
                ao = aop.tile([128, SW], mdt, tag=f"aoT{dd}", name=f"aoT{dd}_{sj}")
                aocur[dd] = ao
                pending = emit_scores(dd, sj, 0, qt)
                for i in range(last + 1):
                    nxt = emit_scores(dd, sj, i + 1, qt) if i < last else None
                    # pop BEFORE the tail: PE is strict FIFO, so filler queued
                    # here executes during the exp latency that gates the PV
                    pop_filler(1)
                    emit_tail(dd, sj, i, pending, pv0, pv1, last)
                    pending = nxt
                emit_norm(dd, sj, pv0, 0)
                emit_norm(dd, sj, pv1, 1)
                return ao

            def emit_wo(sj, ltt, ao_tiles):
                # one token tile, both 512-col halves in one 2-bank psum tile
                tt = 4 * sj + ltt
                tok = slice(128 * ltt, 128 * (ltt + 1))
                ps = ps2.tile([128, 2 * SW], F32, tag="sc", name=f"o{tt}")
                for ee in range(2):
                    for dd in range(4):
                        nc.tensor.matmul(
                            ps[:, SW * ee:SW * (ee + 1)],
                            mc(ao_tiles[dd][:, tok]), mc(wot[:, dd, SW * ee:SW * (ee + 1)]),
                            start=(dd == 0), stop=(dd == 3))
                st = sp3.tile([128, 2 * SW], F32, tag="st", name=f"st{tt}")
                nc.vector.tensor_copy(st[:], ps[:])
                nc.sync.dma_start(out_d[128 * tt:128 * (tt + 1), :], st[:])

            def queue_wo(sj, ltt, ao_tiles):
                # same as emit_wo, but as filler closures: the last swath has
                # no projection filler, so spread wo into its exp bubbles
                tt = 4 * sj + ltt
                tok = slice(128 * ltt, 128 * (ltt + 1))
                box = {}

                def mmstep(ee, dd2, box=box):
                    if ee == 0 and dd2 == 0:
                        box["ps"] = ps2.tile([128, 2 * SW], F32, tag="sc",
                                             name=f"o{tt}")
                    nc.tensor.matmul(
                        box["ps"][:, SW * ee:SW * (ee + 1)],
                        mc(ao_tiles[dd2][:, tok]),
                        mc(wot[:, dd2, SW * ee:SW * (ee + 1)]),
                        start=(dd2 == 0), stop=(dd2 == 3))

                def fin(box=box):
                    st = sp3.tile([128, 2 * SW], F32, tag="st", name=f"st{tt}")
                    nc.vector.tensor_copy(st[:], box["ps"][:])
                    nc.sync.dma_start(out_d[128 * tt:128 * (tt + 1), :], st[:])

                for ee in range(2):
                    for dd2 in range(4):
                        filler.append(lambda ee=ee, dd2=dd2: mmstep(ee, dd2))
                filler.append(fin)

            # ---------------- weave ----------------
            # first x swath in two chunks woven with the rest of wqt, so the
            # opening projection steps start after ~1/4 of the prologue bytes
            nc.sync.dma_start(xs[0][:, 0:2], xT_r[:, 0:2, 0:SW])
            nc.sync.dma_start(wqt[:, 2:5], wqT_r[:, 2:5])
            nc.sync.dma_start(xs[0][:, 2:5], xT_r[:, 2:5, 0:SW])
            nc.sync.dma_start(wqt[:, 5:ND], wqT_r[:, 5:ND])
            nc.sync.dma_start(xs[0][:, 5:ND], xT_r[:, 5:ND, 0:SW])
            nc.sync.dma_start(bqT[:], bqT_d[:])
            nc.sync.dma_start(cmT[:], cmT_d[:])
            nc.sync.dma_start(id128[:], id_d[:])
            nc.sync.dma_start(wkt[:, 0:4], wkT_r[:, 0:4])
            nc.sync.dma_start(wkt[:, 4:ND], wkT_r[:, 4:ND])
            nc.sync.dma_start(xs[1][:], xT_r[:, :, SW:2 * SW])
            nc.sync.dma_start(wvt[:], wvT_r[:])
            nc.sync.dma_start(wot[:], woT_r[:])
            # flush only what swath-0 attention needs up front; q23/k23 stay
            # queued as filler for the first attention blocks
            proj_pair_qk(0, xs[0], "q", 0, 1)
            proj_pair_qk(0, xs[0], "k", 0, 1)
            proj_pair_v(0, xs[0], 0, 1)
            proj_pair_v(0, xs[0], 2, 3)
            pop_filler(len(filler))
            proj_pair_qk(0, xs[0], "q", 2, 3)
            proj_pair_qk(0, xs[0], "k", 2, 3)

            ao_prev = None
            for sj in range(NSW):
                if sj + 2 < NSW:
                    # persistent tiles: no WAR gate, DMA runs as soon as the
                    # queue reaches it
                    nc.sync.dma_start(xs[sj + 2][:],
                                      xT_r[:, :, SW * (sj + 2):SW * (sj + 3)])
                if sj + 1 < NSW:
                    # queue next swath's projections; they emit as filler
                    proj_pair_qk(sj + 1, xs[sj + 1], "q", 0, 1)
                    proj_pair_qk(sj + 1, xs[sj + 1], "q", 2, 3)
                    proj_pair_qk(sj + 1, xs[sj + 1], "k", 0, 1)
                    proj_pair_qk(sj + 1, xs[sj + 1], "k", 2, 3)
                    proj_pair_v(sj + 1, xs[sj + 1], 0, 1)
                    proj_pair_v(sj + 1, xs[sj + 1], 2, 3)
                ao_now = [None] * 4
                for dd in range(4):
                    if sj == NSW - 1 and ao_prev is not None:
                        # last swath has no proj filler; weave wo(sj-1) into
                        # the attention i-loop instead of bursting it after
                        queue_wo(sj - 1, dd, ao_prev)
                    ao_now[dd] = emit_att(dd, sj, qtab[sj][dd])
                    # drain some filler between blocks, plus wo for sj-1
                    pop_filler(3 if sj > 0 else 9)
                    if sj != NSW - 1 and ao_prev is not None:
                        emit_wo(sj - 1, dd, ao_prev)
                ao_prev = ao_now
                pop_filler(len(filler))  # flush: next swath's q/k/v must be ready
            # final swath's wo: emit each tile's dd=0..2 parts first (their ao
            # is ready during dd=3's attention), dd=3 part + copy last, so the
            # PE chews matmuls while the last norm chain completes on
            # vector/gpsimd. Stagger to 3 live chains (ps2 pool depth).
            wops = {}

            def wo_part(ltt, dds, start, stop):
                tok = slice(128 * ltt, 128 * (ltt + 1))
                if ltt not in wops:
                    wops[ltt] = ps2.tile([128, 2 * SW], F32, tag="sc",
                                         name=f"o{4 * (NSW - 1) + ltt}")
                ps = wops[ltt]
                for ee in range(2):
                    for dd2 in dds:
                        nc.tensor.matmul(
                            ps[:, SW * ee:SW * (ee + 1)],
                            mc(ao_prev[dd2][:, tok]),
                            mc(wot[:, dd2, SW * ee:SW * (ee + 1)]),
                            start=(start and dd2 == dds[0]),
                            stop=(stop and dd2 == dds[-1]))

            def wo_fin(ltt, half=None):
                # per-half copies pipeline against the remaining matmuls
                tt = 4 * (NSW - 1) + ltt
                if ltt not in wost:
                    wost[ltt] = sp3.tile([128, 2 * SW], F32, tag="st",
                                         name=f"st{tt}")
                st = wost[ltt]
                halves = range(2) if half is None else (half,)
                for ee in halves:
                    cols = slice(SW * ee, SW * (ee + 1))
                    nc.vector.tensor_copy(st[:, cols], wops[ltt][:, cols])
                if half is None or half == 1:
                    nc.sync.dma_start(out_d[128 * tt:128 * (tt + 1), :], st[:])

            def wo_last(ltt):
                # final dd=3 contribution per half, each half's copy issued
                # immediately so it overlaps the other half's matmul
                tok = slice(128 * ltt, 128 * (ltt + 1))
                for ee in range(2):
                    nc.tensor.matmul(
                        wops[ltt][:, SW * ee:SW * (ee + 1)],
                        mc(ao_prev[3][:, tok]),
                        mc(wot[:, 3, SW * ee:SW * (ee + 1)]),
                        start=False, stop=True)
                    wo_fin(ltt, half=ee)

            wost = {}
            for ltt in (0, 1, 2):
                wo_part(ltt, [0, 1, 2], start=True, stop=False)
            wo_last(0)
            wo_part(3, [0, 1, 2], start=True, stop=False)
            for ltt in (1, 2, 3):
                wo_last(ltt)

    nc.compile()
    return nc


def _get_nc(mode):
    if mode not in _NC_CACHE:
        _NC_CACHE[mode] = _build(mode)
    return _NC_CACHE[mode]


def _causal_mask_tiles():
    # additive triangle for a diagonal 128-block (keep iff q >= p), returned
    # TRANSPOSED for the PE-side mask preload (psum := cmT.T @ I = cm), plus
    # the identity used as the preload's moving operand
    p = np.arange(128)[:, None]
    q = np.arange(128)[None, :]
    cm = np.where(q >= p, np.float32(0.0), np.float32(-1e30)).astype(np.float32)
    return np.ascontiguousarray(cm.T), np.eye(128, dtype=np.float32)


def kernel(x, mask, wq, bq, wk, bk, wv, bv, wo, bo):
    x = np.asarray(x, dtype=np.float32)
    wq = np.asarray(wq, dtype=np.float32)
    bq = np.asarray(bq, dtype=np.float32)
    wk = np.asarray(wk, dtype=np.float32)
    wv = np.asarray(wv, dtype=np.float32)
    bv = np.asarray(bv, dtype=np.float32)
    wo = np.asarray(wo, dtype=np.float32)
    bo = np.asarray(bo, dtype=np.float32)
    # mask is the causal tril (hardcoded in the kernel); bk cancels in softmax

    nc = _get_nc(MODE)
    _, np_dt = _mm_dt(MODE)

    cmT, id128 = _causal_mask_tiles()
    in_maps = []
    for c in range(8):
        b, hg = c // 2, c % 2
        rows = slice(DPC * hg, DPC * (hg + 1))
        in_maps.append({
            "xT": np.ascontiguousarray(x[b].T).astype(np_dt),
            "wqT": np.ascontiguousarray(wq[rows].T).astype(np_dt),
            "wkT": np.ascontiguousarray(wk[rows].T).astype(np_dt),
            "wvT": np.ascontiguousarray(wv[rows].T).astype(np_dt),
            "woT": np.ascontiguousarray(wo[:, rows].T).astype(np_dt),
            "bqT": np.ascontiguousarray(bq[rows].reshape(4, 128).T).astype(np.float32),
            "cmT": cmT.astype(np_dt),
            "id128": id128.astype(np_dt),
        })

    res = run_bass_kernel_spmd(nc, in_maps, list(range(8))).results

    corr = (wo @ bv) + bo  # bv commutes through softmax-normalized attention
    out = np.empty((B, S, D), dtype=np.float32)
    for b in range(B):
        out[b] = res[2 * b]["out"] + res[2 * b + 1]["out"] + corr
    return out



# revision 57
# speedup vs baseline: 1.0005x; 1.0005x over previous
"""Multi-head causal attention (B=4, S=2048, D=1024, H=16, Hd=64) on 8 trn2 cores.

Sharding: data-parallel over batch (4) x tensor-parallel over heads (2 groups
of 8 heads). Core c handles batch c//2 and heads 8*(c%2)..8*(c%2)+7:
  - wq/wk/wv column-parallel (each core owns 512 of the 1024 output dims),
  - wo row-parallel (partial outputs summed on host).

Device-side per core:
  phase 1: qT/kT (transposed, [dq,S]) and v (natural, [S,hd]) projections
  phase 2: per head-pair d, q-swath j: scoresT = kT.T-chunk @ qT-swath (row-
           tiled pair of K=64 matmuls), causal additive mask on diagonal
           tiles, exp on ACT (no max subtraction: scores are O(1), exp is
           safe), PV matmul with a ones-column appended to v so the softmax
           denominator falls out of the same matmul, then normalize.
  phase 3: out_partial = attnT.T @ woT  (row-parallel wo)

Host side: shard/transposes, pair-sum of partials, + wo@bv + bo correction
(bk provably cancels in softmax; bv commutes to a constant because softmax
rows sum to 1).

Math note: softmax computed without max-subtraction (scores ~ N(0,1), exp
overflow impossible in fp32); masked entries get -1e30 pre-exp -> exp = 0.
"""
import sys

sys.path.insert(0, "/opt/trn_rl_repo")

import numpy as np

from concourse import bacc, mybir, tile
from concourse.bass_utils import run_bass_kernel_spmd

B, S, D = 4, 2048, 1024
H, HD = 16, 64
HPC = 8        # heads per core
DPC = HPC * HD  # 512 projection dims per core
SW = 512       # q swath width
NSW = S // SW  # 4
NT = S // 128  # 16 token tiles
ND = D // 128  # 8 contraction chunks

# matmul dtype mode: "f32" (exact, 4x slow), "f32r" (full speed, ~tf32ish),
# "bf16" (full speed, least precise, half DMA/SBUF)
MODE = "bf16"

F32 = mybir.dt.float32
EXPF = mybir.ActivationFunctionType.Exp

_NC_CACHE = {}


def _mm_dt(mode):
    import ml_dtypes
    if mode == "bf16":
        return mybir.dt.bfloat16, ml_dtypes.bfloat16
    if mode in ("f32r", "f32r_hi"):
        # float32r: fp32 storage, PE reads reduced mantissa at full rate.
        # np-side arrays stay fp32.
        return mybir.dt.float32r, np.float32
    return F32, np.float32


def _build(mode):
    mdt, _ = _mm_dt(mode)
    # PV-stage dtype: bf16 operands (exp weights + v) halve SBUF at a
    # ~2e-3 rel-err cost; f32r_hi keeps them f32r (~3.5e-4) with tighter
    # buffer counts to fit SBUF.
    pdt = mybir.dt.bfloat16 if mode == "f32r" else mdt
    hi = mode != "f32r"

    def mc(ap):
        return ap

    nc = bacc.Bacc("TRN2", target_bir_lowering=False, debug=False, num_devices=8)

    xT_d = nc.dram_tensor("xT", [D, S], mdt, kind="ExternalInput").ap()
    wqT_d = nc.dram_tensor("wqT", [D, DPC], mdt, kind="ExternalInput").ap()
    wkT_d = nc.dram_tensor("wkT", [D, DPC], mdt, kind="ExternalInput").ap()
    wvT_d = nc.dram_tensor("wvT", [D, DPC], mdt, kind="ExternalInput").ap()
    woT_d = nc.dram_tensor("woT", [DPC, D], mdt, kind="ExternalInput").ap()
    bqT_d = nc.dram_tensor("bqT", [128, 4], F32, kind="ExternalInput").ap()
    cmT_d = nc.dram_tensor("cmT", [128, 128], mdt, kind="ExternalInput").ap()
    id_d = nc.dram_tensor("id128", [128, 128], mdt, kind="ExternalInput").ap()
    out_d = nc.dram_tensor("out", [S, D], F32, kind="ExternalOutput").ap()

    xT_r = xT_d.rearrange("(c p) s -> p c s", p=128)
    wqT_r = wqT_d.rearrange("(c p) n -> p c n", p=128)
    wkT_r = wkT_d.rearrange("(c p) n -> p c n", p=128)
    wvT_r = wvT_d.rearrange("(c p) n -> p c n", p=128)
    woT_r = woT_d.rearrange("(c p) n -> p c n", p=128)

    with tile.TileContext(nc) as tc:
        with (
            tc.tile_pool(name="pers", bufs=1) as pp,
            tc.tile_pool(name="qts", bufs=2) as qp,
            tc.tile_pool(name="aots", bufs=3) as aop,
            tc.tile_pool(name="xp", bufs=1 if hi else 3) as xp,
            tc.tile_pool(name="exp", bufs=3 if hi else 5) as ep,
            tc.tile_pool(name="rp", bufs=2) as rp,
            tc.tile_pool(name="stp", bufs=4) as sp3,
            tc.tile_pool(name="scp", bufs=3, space="PSUM") as ps2,
            tc.tile_pool(name="pvp", bufs=1, space="PSUM") as pvp,
        ):
            kT = [pp.tile([128, S], mdt, tag=f"kT{d}", name=f"kT{d}") for d in range(4)]
            v3 = [pp.tile([128, HPC, HD + 1], pdt, tag=f"v{t}", name=f"v{t}") for t in range(NT)]
            wqt = pp.tile([128, ND, DPC], mdt, tag="wqt", name="wqt")
            wkt = pp.tile([128, ND, DPC], mdt, tag="wkt", name="wkt")
            wvt = pp.tile([128, ND, DPC], mdt, tag="wvt", name="wvt")
            wot = pp.tile([128, 4, D], mdt, tag="wot", name="wot")
            bqT = pp.tile([128, 4], F32, tag="bqT", name="bqT")
            zb = pp.tile([128, 1], F32, tag="zb", name="zb")
            cmT = pp.tile([128, 128], mdt, tag="cmT", name="cmT")
            id128 = pp.tile([128, 128], mdt, tag="id128", name="id128")
            # wqt first: the opening projections only need it (+ xsw(0));
            # chunked so the first proj steps start after ~1/4 of the bytes.
            # bqT/cmT/id128 are only consumed from the first drain/attention
            # on, so they queue behind the critical path.
            nc.sync.dma_start(wqt[:, 0:2], wqT_r[:, 0:2])
            nc.vector.memset(zb[:], 0.0)
            for t in range(NT):
                # ones column for the in-PV softmax denominator; v3 tiles are
                # persistent so this is set once, drains only write [0:HD]
                nc.vector.memset(v3[t][:, :, HD:HD + 1], 1.0)

            qtab = {sj: [None] * 4 for sj in range(NSW)}  # per (sj, dd) qT tile
            aocur = [None] * 4   # per-dd current swath attnT tile

            # persistent per-swath x tiles: unique tags mean no pool-rotation
            # WAR gates, so every x DMA can run during the prologue
            xs = [pp.tile([128, ND, SW], mdt, tag=f"xsw{j}", name=f"xsw{j}")
                  for j in range(NSW)]

            filler = []  # FIFO of emission closures (each ~2 matmuls of filler)

            def proj_pair_qk(sj, xsw, which, da, db):
                # two projection outputs share one 2-bank psum tile; dk-steps
                # are queued as filler closures woven into attention i-loops
                wt = wqt if which == "q" else wkt
                box = {}

                def step(dk, box=box):
                    if dk == 0:
                        box["ps"] = ps2.tile([128, 2 * SW], F32, tag="sc", name=f"p{which}{sj}_{da}")
                    ps = box["ps"]
                    nc.tensor.matmul(
                        ps[:, 0:SW], mc(wt[:, dk, 128 * da:128 * da + 128]),
                        mc(xsw[:, dk, :]), start=(dk == 0), stop=(dk == ND - 1))
                    nc.tensor.matmul(
                        ps[:, SW:2 * SW], mc(wt[:, dk, 128 * db:128 * db + 128]),
                        mc(xsw[:, dk, :]), start=(dk == 0), stop=(dk == ND - 1))

                def drain(box=box):
                    ps = box["ps"]
                    cols = slice(SW * sj, SW * (sj + 1))
                    if which == "q":
                        for half, dd in ((0, da), (1, db)):
                            qt = qp.tile([128, SW], mdt, tag=f"qT{dd}", name=f"qT{dd}_{sj}")
                            nc.vector.tensor_scalar_add(
                                qt[:], ps[:, SW * half:SW * (half + 1)], bqT[:, dd:dd + 1])
                            qtab[sj][dd] = qt
                    else:
                        for half, dd in ((0, da), (1, db)):
                            nc.vector.tensor_copy(kT[dd][:, cols], ps[:, SW * half:SW * (half + 1)])

                for dk in range(ND):
                    filler.append(lambda dk=dk: step(dk))
                filler.append(drain)

            def proj_pair_v(sj, xsw, ta, tb):
                box = {}

                def step(dk, box=box):
                    if dk == 0:
                        box["ps"] = ps2.tile([128, 2 * SW], F32, tag="sc", name=f"pv{sj}_{ta}")
                    ps = box["ps"]
                    nc.tensor.matmul(
                        ps[:, 0:SW], mc(xsw[:, dk, 128 * ta:128 * ta + 128]),
                        mc(wvt[:, dk, :]), start=(dk == 0), stop=(dk == ND - 1))
                    nc.tensor.matmul(
                        ps[:, SW:2 * SW], mc(xsw[:, dk, 128 * tb:128 * tb + 128]),
                        mc(wvt[:, dk, :]), start=(dk == 0), stop=(dk == ND - 1))

                def drain(box=box):
                    ps = box["ps"]
                    for half, tloc in ((0, ta), (1, tb)):
                        t = 4 * sj + tloc
                        nc.vector.tensor_copy(
                            v3[t][:, :, 0:HD],
                            ps[:, SW * half:SW * (half + 1)].rearrange("p (h e) -> p h e", h=HPC))

                for dk in range(ND):
                    filler.append(lambda dk=dk: step(dk))
                filler.append(drain)

            def pop_filler(n):
                for _ in range(n):
                    if not filler:
                        return
                    filler.pop(0)()

            def emit_scores(dd, sj, i, qt):
                krows = slice(128 * i, 128 * (i + 1))
                # diagonal key tiles: queries below c0 are fully masked, skip
                # their score columns
                c0 = 128 * (i - 4 * sj) if i >= 4 * sj else 0
                ps = ps2.tile([128, 2 * SW], F32, tag="sc", name=f"sc{dd}_{sj}_{i}")
                if c0 == 0 and i < 4 * sj:
                    # off-diagonal tile: plain full-width scores
                    nc.tensor.matmul(ps[:, 0:SW], mc(kT[dd][0:64, krows]),
                                     mc(qt[0:64, :]))
                    nc.tensor.matmul(ps[:, SW:2 * SW], mc(kT[dd][64:128, krows]),
                                     mc(qt[64:128, :]))
                    return ps
                # diagonal tile: preload the additive causal mask into the
                # 128-wide diagonal block via a PE matmul (cmT @ I), then
                # accumulate scores on top; exp can then read psum directly
                # with no vector mask-add in the chain.
                c1 = c0 + 128
                for g in range(2):
                    nc.tensor.matmul(ps[:, g * SW + c0:g * SW + c1], mc(cmT[:]),
                                     mc(id128[:]), start=True, stop=False)
                nc.tensor.matmul(ps[:, c0:c1], mc(kT[dd][0:64, krows]),
                                 mc(qt[0:64, c0:c1]), start=False, stop=True)
                nc.tensor.matmul(ps[:, SW + c0:SW + c1], mc(kT[dd][64:128, krows]),
                                 mc(qt[64:128, c0:c1]), start=False, stop=True)
                if c1 < SW:
                    nc.tensor.matmul(ps[:, c1:SW], mc(kT[dd][0:64, krows]),
                                     mc(qt[0:64, c1:SW]))
                    nc.tensor.matmul(ps[:, SW + c1:2 * SW], mc(kT[dd][64:128, krows]),
                                     mc(qt[64:128, c1:SW]))
                return ps

            def emit_tail(dd, sj, i, ps, pv0, pv1, last):
                h0, h1 = 2 * dd, 2 * dd + 1
                t = i - 4 * sj
                c0 = 128 * t if t >= 0 else 0
                ex = ep.tile([128, 2 * SW], pdt, tag="ex", name=f"ex{dd}_{sj}_{i}")
                if t >= 0:
                    pse = ps[:].rearrange("p (g q) -> p g q", g=2)[:, :, c0:SW]
                    exe = ex[:].rearrange("p (g q) -> p g q", g=2)[:, :, c0:SW]
                    nc.scalar.activation(exe, pse, EXPF, bias=zb[:], scale=0.125)
                else:
                    nc.scalar.activation(ex[:], ps[:], EXPF, bias=zb[:], scale=0.125)
                nc.tensor.matmul(
                    pv0[0:HD + 1, c0:SW], mc(v3[i][:, h0, :]), mc(ex[:, c0:SW]),
                    start=(i == 0), stop=(i == last))
                nc.tensor.matmul(
                    pv1[0:HD + 1, c0:SW], mc(v3[i][:, h1, :]), mc(ex[:, SW + c0:2 * SW]),
                    start=(i == 0), stop=(i == last))

            def emit_norm(dd, sj, pv, hh):
                rb_ = rp.tile([64, SW], F32, tag=f"rb{hh}", name=f"rb{hh}_{dd}_{sj}")
                r_ = rp.tile([1, SW], F32, tag=f"r{hh}", name=f"r{hh}_{dd}_{sj}")
                nc.vector.tensor_copy(rb_[0:1, :], pv[HD:HD + 1, :])
                nc.vector.reciprocal_approx_fast(out=r_[0:1, :], in_=rb_[0:1, :])
                nc.gpsimd.partition_broadcast(rb_[0:64, :], r_[0:1, :])
                if hh == 0:
                    nc.vector.tensor_mul(aocur[dd][0:64, :], pv[0:64, :], rb_[0:64, :])
                else:
                    # DVE cross-quadrant write: shift h1's normalized output up
                    # to partitions 64:128 without a DMA
                    nc.vector.tensor_mul(aocur[dd][64:128, :], pv[0:64, :], rb_[0:64, :])

            def emit_att(dd, sj, qt):
                last = 4 * sj + 3
                pv0 = pvp.tile([128, SW], F32, tag="pv0", name=f"pvh0_{dd}_{sj}")
                pv1 = pvp.tile([128, SW], F32, tag="pv1", name=f"pvh1_{dd}_{sj}")
                ao = aop.tile([128, SW], mdt, tag=f"aoT{dd}", name=f"aoT{dd}_{sj}")
                aocur[dd] = ao
                pending = emit_scores(dd, sj, 0, qt)
                for i in range(last + 1):
                    nxt = emit_scores(dd, sj, i + 1, qt) if i < last else None
                    # pop BEFORE the tail: PE is strict FIFO, so filler queued
                    # here executes during the exp latency that gates the PV
                    pop_filler(1)
                    emit_tail(dd, sj, i, pending, pv0, pv1, last)
                    pending = nxt
                emit_norm(dd, sj, pv0, 0)
                emit_norm(dd, sj, pv1, 1)
                return ao

            def emit_wo(sj, ltt, ao_tiles):
                # one token tile, both 512-col halves in one 2-bank psum tile
                tt = 4 * sj + ltt
                tok = slice(128 * ltt, 128 * (ltt + 1))
                ps = ps2.tile([128, 2 * SW], F32, tag="sc", name=f"o{tt}")
                for ee in range(2):
                    for dd in range(4):
                        nc.tensor.matmul(
                            ps[:, SW * ee:SW * (ee + 1)],
                            mc(ao_tiles[dd][:, tok]), mc(wot[:, dd, SW * ee:SW * (ee + 1)]),
                            start=(dd == 0), stop=(dd == 3))
                st = sp3.tile([128, 2 * SW], F32, tag="st", name=f"st{tt}")
                nc.vector.tensor_copy(st[:], ps[:])
                nc.sync.dma_start(out_d[128 * tt:128 * (tt + 1), :], st[:])

            def queue_wo(sj, ltt, ao_tiles):
                # same as emit_wo, but as filler closures: the last swath has
                # no projection filler, so spread wo into its exp bubbles
                tt = 4 * sj + ltt
                tok = slice(128 * ltt, 128 * (ltt + 1))
                box = {}

                def mmstep(ee, dd2, box=box):
                    if ee == 0 and dd2 == 0:
                        box["ps"] = ps2.tile([128, 2 * SW], F32, tag="sc",
                                             name=f"o{tt}")
                    nc.tensor.matmul(
                        box["ps"][:, SW * ee:SW * (ee + 1)],
                        mc(ao_tiles[dd2][:, tok]),
                        mc(wot[:, dd2, SW * ee:SW * (ee + 1)]),
                        start=(dd2 == 0), stop=(dd2 == 3))

                def fin(box=box):
                    st = sp3.tile([128, 2 * SW], F32, tag="st", name=f"st{tt}")
                    nc.vector.tensor_copy(st[:], box["ps"][:])
                    nc.sync.dma_start(out_d[128 * tt:128 * (tt + 1), :], st[:])

                for ee in range(2):
                    for dd2 in range(4):
                        filler.append(lambda ee=ee, dd2=dd2: mmstep(ee, dd2))
                filler.append(fin)

            # ---------------- weave ----------------
            # first x swath in two chunks woven with the rest of wqt, so the
            # opening projection steps start after ~1/4 of the prologue bytes
            nc.sync.dma_start(xs[0][:, 0:2], xT_r[:, 0:2, 0:SW])
            nc.sync.dma_start(wqt[:, 2:5], wqT_r[:, 2:5])
            nc.sync.dma_start(xs[0][:, 2:5], xT_r[:, 2:5, 0:SW])
            nc.sync.dma_start(wqt[:, 5:ND], wqT_r[:, 5:ND])
            nc.sync.dma_start(xs[0][:, 5:ND], xT_r[:, 5:ND, 0:SW])
            nc.sync.dma_start(bqT[:], bqT_d[:])
            nc.sync.dma_start(cmT[:], cmT_d[:])
            nc.sync.dma_start(id128[:], id_d[:])
            nc.sync.dma_start(wkt[:, 0:4], wkT_r[:, 0:4])
            nc.sync.dma_start(wkt[:, 4:ND], wkT_r[:, 4:ND])
            nc.sync.dma_start(xs[1][:], xT_r[:, :, SW:2 * SW])
            nc.sync.dma_start(wvt[:], wvT_r[:])
            nc.sync.dma_start(wot[:], woT_r[:])
            # flush only what swath-0 attention needs up front; q23/k23 stay
            # queued as filler for the first attention blocks
            proj_pair_qk(0, xs[0], "q", 0, 1)
            proj_pair_qk(0, xs[0], "k", 0, 1)
            proj_pair_v(0, xs[0], 0, 1)
            proj_pair_v(0, xs[0], 2, 3)
            pop_filler(len(filler))
            proj_pair_qk(0, xs[0], "q", 2, 3)
            proj_pair_qk(0, xs[0], "k", 2, 3)

            ao_prev = None
            for sj in range(NSW):
                if sj + 2 < NSW:
                    # persistent tiles: no WAR gate, DMA runs as soon as the
                    # queue reaches it
                    nc.sync.dma_start(xs[sj + 2][:],
                                      xT_r[:, :, SW * (sj + 2):SW * (sj + 3)])
                if sj + 1 < NSW:
                    # queue next swath's projections; they emit as filler
                    proj_pair_qk(sj + 1, xs[sj + 1], "q", 0, 1)
                    proj_pair_qk(sj + 1, xs[sj + 1], "q", 2, 3)
                    proj_pair_qk(sj + 1, xs[sj + 1], "k", 0, 1)
                    proj_pair_qk(sj + 1, xs[sj + 1], "k", 2, 3)
                    proj_pair_v(sj + 1, xs[sj + 1], 0, 1)
                    proj_pair_v(sj + 1, xs[sj + 1], 2, 3)
                ao_now = [None] * 4
                for dd in range(4):
                    if sj == NSW - 1 and ao_prev is not None:
                        # last swath has no proj filler; weave wo(sj-1) into
                        # the attention i-loop instead of bursting it after
                        queue_wo(sj - 1, dd, ao_prev)
                    ao_now[dd] = emit_att(dd, sj, qtab[sj][dd])
                    # drain some filler between blocks, plus wo for sj-1
                    pop_filler(3 if sj > 0 else 9)
                    if sj != NSW - 1 and ao_prev is not None:
                        emit_wo(sj - 1, dd, ao_prev)
                ao_prev = ao_now
                pop_filler(len(filler))  # flush: next swath's q/k/v must be ready
            # final swath's wo: emit each tile's dd=0..2 parts first (their ao
            # is ready during dd=3's attention), dd=3 part + copy last, so the
            # PE chews matmuls while the last norm chain completes on
            # vector/gpsimd. Stagger to 3 live chains (ps2 pool depth).
            wops = {}

            def wo_part(ltt, dds, start, stop):
                tok = slice(128 * ltt, 128 * (ltt + 1))
                if ltt not in wops:
                    wops[ltt] = ps2.tile([128, 2 * SW], F32, tag="sc",
                                         name=f"o{4 * (NSW - 1) + ltt}")
                ps = wops[ltt]
                for ee in range(2):
                    for dd2 in dds:
                        nc.tensor.matmul(
                            ps[:, SW * ee:SW * (ee + 1)],
                            mc(ao_prev[dd2][:, tok]),
                            mc(wot[:, dd2, SW * ee:SW * (ee + 1)]),
                            start=(start and dd2 == dds[0]),
                            stop=(stop and dd2 == dds[-1]))

            def wo_fin(ltt, half=None):
                # per-half copies pipeline against the remaining matmuls
                tt = 4 * (NSW - 1) + ltt
                if ltt not in wost:
                    wost[ltt] = sp3.tile([128, 2 * SW], F32, tag="st",
                                         name=f"st{tt}")
                st = wost[ltt]
                halves = range(2) if half is None else (half,)
                for ee in halves:
                    cols = slice(SW * ee, SW * (ee + 1))
                    nc.vector.tensor_copy(st[:, cols], wops[ltt][:, cols])
                if half is None or half == 1:
                    nc.sync.dma_start(out_d[128 * tt:128 * (tt + 1), :], st[:])

            def wo_last(ltt):
                # final dd=3 contribution per half, each half's copy issued
                # immediately so it overlaps the other half's matmul
                tok = slice(128 * ltt, 128 * (ltt + 1))
                for ee in range(2):
                    nc.tensor.matmul(
                        wops[ltt][:, SW * ee:SW * (ee + 1)],
                        mc(ao_prev[3][:, tok]),
                        mc(wot[:, 3, SW * ee:SW * (ee + 1)]),
                        start=False, stop=True)
                    wo_fin(ltt, half=ee)

            wost = {}
            for ltt in (0, 1, 2):
                wo_part(ltt, [0, 1, 2], start=True, stop=False)
            wo_last(0)
            wo_part(3, [0, 1, 2], start=True, stop=False)
            for ltt in (1, 2, 3):
                wo_last(ltt)

    nc.compile()
    return nc


def _get_nc(mode):
    if mode not in _NC_CACHE:
        _NC_CACHE[mode] = _build(mode)
    return _NC_CACHE[mode]


def _causal_mask_tiles():
    # additive triangle for a diagonal 128-block (keep iff q >= p), returned
    # TRANSPOSED for the PE-side mask preload (psum := cmT.T @ I = cm), plus
    # the identity used as the preload's moving operand
    p = np.arange(128)[:, None]
    q = np.arange(128)[None, :]
    cm = np.where(q >= p, np.float32(0.0), np.float32(-1e30)).astype(np.float32)
    return np.ascontiguousarray(cm.T), np.eye(128, dtype=np.float32)


def kernel(x, mask, wq, bq, wk, bk, wv, bv, wo, bo):
    x = np.asarray(x, dtype=np.float32)
    wq = np.asarray(wq, dtype=np.float32)
    bq = np.asarray(bq, dtype=np.float32)
    wk = np.asarray(wk, dtype=np.float32)
    wv = np.asarray(wv, dtype=np.float32)
    bv = np.asarray(bv, dtype=np.float32)
    wo = np.asarray(wo, dtype=np.float32)
    bo = np.asarray(bo, dtype=np.float32)
    # mask is the causal tril (hardcoded in the kernel); bk cancels in softmax

    nc = _get_nc(MODE)
    _, np_dt = _mm_dt(MODE)

    cmT, id128 = _causal_mask_tiles()
    in_maps = []
    for c in range(8):
        b, hg = c // 2, c % 2
        rows = slice(DPC * hg, DPC * (hg + 1))
        in_maps.append({
            "xT": np.ascontiguousarray(x[b].T).astype(np_dt),
            "wqT": np.ascontiguousarray(wq[rows].T).astype(np_dt),
            "wkT": np.ascontiguousarray(wk[rows].T).astype(np_dt),
            "wvT": np.ascontiguousarray(wv[rows].T).astype(np_dt),
            "woT": np.ascontiguousarray(wo[:, rows].T).astype(np_dt),
            "bqT": np.ascontiguousarray(bq[rows].reshape(4, 128).T).astype(np.float32),
            "cmT": cmT.astype(np_dt),
            "id128": id128.astype(np_dt),
        })

    res = run_bass_kernel_spmd(nc, in_maps, list(range(8))).results

    corr = (wo @ bv) + bo  # bv commutes through softmax-normalized attention
    out = np.empty((B, S, D), dtype=np.float32)
    for b in range(B):
        out[b] = res[2 * b]["out"] + res[2 * b + 1]["out"] + corr
    return out



# revision 60
# speedup vs baseline: 1.1834x; 1.1828x over previous
"""Multi-head causal attention (B=4, S=2048, D=1024, H=16, Hd=64) on 8 trn2 cores.

Sharding: data-parallel over batch (4) x tensor-parallel over heads (2 groups
of 8 heads). Core c handles batch c//2 and heads 8*(c%2)..8*(c%2)+7:
  - wq/wk/wv column-parallel (each core owns 512 of the 1024 output dims),
  - wo row-parallel (partial outputs summed on host).

Device-side per core:
  phase 1: qT/kT (transposed, [dq,S]) and v (natural, [S,hd]) projections
  phase 2: per head-pair d, q-swath j: scoresT = kT.T-chunk @ qT-swath (row-
           tiled pair of K=64 matmuls), causal additive mask on diagonal
           tiles, exp on ACT (no max subtraction: scores are O(1), exp is
           safe), PV matmul with a ones-column appended to v so the softmax
           denominator falls out of the same matmul, then normalize.
  phase 3: out_partial = attnT.T @ woT  (row-parallel wo)

Host side: shard/transposes, pair-sum of partials, + wo@bv + bo correction
(bk provably cancels in softmax; bv commutes to a constant because softmax
rows sum to 1).

Math note: softmax computed without max-subtraction (scores ~ N(0,1), exp
overflow impossible in fp32); masked entries get -1e30 pre-exp -> exp = 0.
"""
import sys

sys.path.insert(0, "/opt/trn_rl_repo")

import numpy as np

from concourse import bacc, mybir, tile
from concourse.bass_utils import run_bass_kernel_spmd

B, S, D = 4, 2048, 1024
H, HD = 16, 64
HPC = 8        # heads per core
DPC = HPC * HD  # 512 projection dims per core
SW = 512       # q swath width
NSW = S // SW  # 4
NT = S // 128  # 16 token tiles
ND = D // 128  # 8 contraction chunks

# matmul dtype mode: "f32" (exact, 4x slow), "f32r" (full speed, ~tf32ish),
# "bf16" (full speed, least precise, half DMA/SBUF)
MODE = "bf16"

F32 = mybir.dt.float32
EXPF = mybir.ActivationFunctionType.Exp

_NC_CACHE = {}


def _mm_dt(mode):
    import ml_dtypes
    if mode == "bf16":
        return mybir.dt.bfloat16, ml_dtypes.bfloat16
    if mode in ("f32r", "f32r_hi"):
        # float32r: fp32 storage, PE reads reduced mantissa at full rate.
        # np-side arrays stay fp32.
        return mybir.dt.float32r, np.float32
    return F32, np.float32


def _build(mode):
    mdt, _ = _mm_dt(mode)
    # PV-stage dtype: bf16 operands (exp weights + v) halve SBUF at a
    # ~2e-3 rel-err cost; f32r_hi keeps them f32r (~3.5e-4) with tighter
    # buffer counts to fit SBUF.
    pdt = mybir.dt.bfloat16 if mode == "f32r" else mdt
    hi = mode != "f32r"

    def mc(ap):
        return ap

    nc = bacc.Bacc("TRN2", target_bir_lowering=False, debug=False, num_devices=8)

    xT_d = nc.dram_tensor("xT", [D, S], mdt, kind="ExternalInput").ap()
    wqT_d = nc.dram_tensor("wqT", [D, DPC], mdt, kind="ExternalInput").ap()
    wkT_d = nc.dram_tensor("wkT", [D, DPC], mdt, kind="ExternalInput").ap()
    wvT_d = nc.dram_tensor("wvT", [D, DPC], mdt, kind="ExternalInput").ap()
    woT_d = nc.dram_tensor("woT", [DPC, D], mdt, kind="ExternalInput").ap()
    bqT_d = nc.dram_tensor("bqT", [128, 4], F32, kind="ExternalInput").ap()
    cmT_d = nc.dram_tensor("cmT", [128, 128], mdt, kind="ExternalInput").ap()
    id_d = nc.dram_tensor("id128", [128, 128], mdt, kind="ExternalInput").ap()
    out_d = nc.dram_tensor("out", [S, D], F32, kind="ExternalOutput").ap()

    xT_r = xT_d.rearrange("(c p) s -> p c s", p=128)
    wqT_r = wqT_d.rearrange("(c p) n -> p c n", p=128)
    wkT_r = wkT_d.rearrange("(c p) n -> p c n", p=128)
    wvT_r = wvT_d.rearrange("(c p) n -> p c n", p=128)
    woT_r = woT_d.rearrange("(c p) n -> p c n", p=128)

    with tile.TileContext(nc) as tc:
        with (
            tc.tile_pool(name="pers", bufs=1) as pp,
            tc.tile_pool(name="qts", bufs=2) as qp,
            tc.tile_pool(name="aots", bufs=3) as aop,
            tc.tile_pool(name="xp", bufs=1 if hi else 3) as xp,
            tc.tile_pool(name="exp", bufs=3 if hi else 5) as ep,
            tc.tile_pool(name="rp", bufs=2) as rp,
            tc.tile_pool(name="stp", bufs=4) as sp3,
            tc.tile_pool(name="scp", bufs=3, space="PSUM") as ps2,
            tc.tile_pool(name="pvp", bufs=1, space="PSUM") as pvp,
        ):
            kT = [pp.tile([128, S], mdt, tag=f"kT{d}", name=f"kT{d}") for d in range(4)]
            v3 = [pp.tile([128, HPC, HD + 1], pdt, tag=f"v{t}", name=f"v{t}") for t in range(NT)]
            wqt = pp.tile([128, ND, DPC], mdt, tag="wqt", name="wqt")
            wkt = pp.tile([128, ND, DPC], mdt, tag="wkt", name="wkt")
            wvt = pp.tile([128, ND, DPC], mdt, tag="wvt", name="wvt")
            wot = pp.tile([128, 4, D], mdt, tag="wot", name="wot")
            bqT = pp.tile([128, 4], F32, tag="bqT", name="bqT")
            zb = pp.tile([128, 1], F32, tag="zb", name="zb")
            ones8 = pp.tile([128, HPC], F32, tag="ones8", name="ones8")
            cmT = pp.tile([128, 128], mdt, tag="cmT", name="cmT")
            id128 = pp.tile([128, 128], mdt, tag="id128", name="id128")
            # wqt first: the opening projections only need it (+ xsw(0));
            # chunked so the first proj steps start after ~1/4 of the bytes.
            # bqT/cmT/id128 are only consumed from the first drain/attention
            # on, so they queue behind the critical path.
            nc.sync.dma_start(wqt[:, 0:2], wqT_r[:, 0:2])
            nc.vector.memset(zb[:], 0.0)
            nc.vector.memset(ones8[:], 1.0)

            qtab = {sj: [None] * 4 for sj in range(NSW)}  # per (sj, dd) qT tile
            aocur = [None] * 4   # per-dd current swath attnT tile

            # persistent per-swath x tiles: unique tags mean no pool-rotation
            # WAR gates, so every x DMA can run during the prologue
            xs = [pp.tile([128, ND, SW], mdt, tag=f"xsw{j}", name=f"xsw{j}")
                  for j in range(NSW)]

            filler = []  # FIFO of emission closures (each ~2 matmuls of filler)

            def proj_pair_qk(sj, xsw, which, da, db):
                # two projection outputs share one 2-bank psum tile; dk-steps
                # are queued as filler closures woven into attention i-loops
                wt = wqt if which == "q" else wkt
                box = {}

                def step(dk, box=box):
                    if dk == 0:
                        box["ps"] = ps2.tile([128, 2 * SW], F32, tag="sc", name=f"p{which}{sj}_{da}")
                    ps = box["ps"]
                    nc.tensor.matmul(
                        ps[:, 0:SW], mc(wt[:, dk, 128 * da:128 * da + 128]),
                        mc(xsw[:, dk, :]), start=(dk == 0), stop=(dk == ND - 1))
                    nc.tensor.matmul(
                        ps[:, SW:2 * SW], mc(wt[:, dk, 128 * db:128 * db + 128]),
                        mc(xsw[:, dk, :]), start=(dk == 0), stop=(dk == ND - 1))

                def drain(box=box):
                    ps = box["ps"]
                    cols = slice(SW * sj, SW * (sj + 1))
                    if which == "q":
                        for half, dd in ((0, da), (1, db)):
                            qt = qp.tile([128, SW], mdt, tag=f"qT{dd}", name=f"qT{dd}_{sj}")
                            nc.vector.tensor_scalar_add(
                                qt[:], ps[:, SW * half:SW * (half + 1)], bqT[:, dd:dd + 1])
                            qtab[sj][dd] = qt
                    else:
                        for half, dd in ((0, da), (1, db)):
                            nc.vector.tensor_copy(kT[dd][:, cols], ps[:, SW * half:SW * (half + 1)])

                for dk in range(ND):
                    filler.append(lambda dk=dk: step(dk))
                filler.append(drain)

            def proj_pair_v(sj, xsw, ta, tb):
                box = {}

                def step(dk, box=box):
                    if dk == 0:
                        box["ps"] = ps2.tile([128, 2 * SW], F32, tag="sc", name=f"pv{sj}_{ta}")
                    ps = box["ps"]
                    nc.tensor.matmul(
                        ps[:, 0:SW], mc(xsw[:, dk, 128 * ta:128 * ta + 128]),
                        mc(wvt[:, dk, :]), start=(dk == 0), stop=(dk == ND - 1))
                    nc.tensor.matmul(
                        ps[:, SW:2 * SW], mc(xsw[:, dk, 128 * tb:128 * tb + 128]),
                        mc(wvt[:, dk, :]), start=(dk == 0), stop=(dk == ND - 1))

                def drain(box=box):
                    ps = box["ps"]
                    for half, tloc in ((0, ta), (1, tb)):
                        t = 4 * sj + tloc
                        nc.vector.tensor_copy(
                            v3[t][:, :, 0:HD],
                            ps[:, SW * half:SW * (half + 1)].rearrange("p (h e) -> p h e", h=HPC))
                        nc.vector.tensor_copy(v3[t][:, :, HD:HD + 1].squeeze(), ones8[:])

                for dk in range(ND):
                    filler.append(lambda dk=dk: step(dk))
                filler.append(drain)

            def pop_filler(n):
                for _ in range(n):
                    if not filler:
                        return
                    filler.pop(0)()

            def emit_scores(dd, sj, i, qt):
                krows = slice(128 * i, 128 * (i + 1))
                # diagonal key tiles: queries below c0 are fully masked, skip
                # their score columns
                c0 = 128 * (i - 4 * sj) if i >= 4 * sj else 0
                ps = ps2.tile([128, 2 * SW], F32, tag="sc", name=f"sc{dd}_{sj}_{i}")
                if c0 == 0 and i < 4 * sj:
                    # off-diagonal tile: plain full-width scores
                    nc.tensor.matmul(ps[:, 0:SW], mc(kT[dd][0:64, krows]),
                                     mc(qt[0:64, :]))
                    nc.tensor.matmul(ps[:, SW:2 * SW], mc(kT[dd][64:128, krows]),
                                     mc(qt[64:128, :]))
                    return ps
                # diagonal tile: preload the additive causal mask into the
                # 128-wide diagonal block via a PE matmul (cmT @ I), then
                # accumulate scores on top; exp can then read psum directly
                # with no vector mask-add in the chain.
                c1 = c0 + 128
                for g in range(2):
                    nc.tensor.matmul(ps[:, g * SW + c0:g * SW + c1], mc(cmT[:]),
                                     mc(id128[:]), start=True, stop=False)
                nc.tensor.matmul(ps[:, c0:c1], mc(kT[dd][0:64, krows]),
                                 mc(qt[0:64, c0:c1]), start=False, stop=True)
                nc.tensor.matmul(ps[:, SW + c0:SW + c1], mc(kT[dd][64:128, krows]),
                                 mc(qt[64:128, c0:c1]), start=False, stop=True)
                if c1 < SW:
                    nc.tensor.matmul(ps[:, c1:SW], mc(kT[dd][0:64, krows]),
                                     mc(qt[0:64, c1:SW]))
                    nc.tensor.matmul(ps[:, SW + c1:2 * SW], mc(kT[dd][64:128, krows]),
                                     mc(qt[64:128, c1:SW]))
                return ps

            def emit_tail(dd, sj, i, ps, pv0, pv1, last):
                h0, h1 = 2 * dd, 2 * dd + 1
                t = i - 4 * sj
                c0 = 128 * t if t >= 0 else 0
                ex = ep.tile([128, 2 * SW], pdt, tag="ex", name=f"ex{dd}_{sj}_{i}")
                if t >= 0:
                    pse = ps[:].rearrange("p (g q) -> p g q", g=2)[:, :, c0:SW]
                    exe = ex[:].rearrange("p (g q) -> p g q", g=2)[:, :, c0:SW]
                    nc.scalar.activation(exe, pse, EXPF, bias=zb[:], scale=0.125)
                else:
                    nc.scalar.activation(ex[:], ps[:], EXPF, bias=zb[:], scale=0.125)
                nc.tensor.matmul(
                    pv0[0:HD + 1, c0:SW], mc(v3[i][:, h0, :]), mc(ex[:, c0:SW]),
                    start=(i == 0), stop=(i == last))
                nc.tensor.matmul(
                    pv1[0:HD + 1, c0:SW], mc(v3[i][:, h1, :]), mc(ex[:, SW + c0:2 * SW]),
                    start=(i == 0), stop=(i == last))

            def emit_norm(dd, sj, pv, hh):
                rb_ = rp.tile([64, SW], F32, tag=f"rb{hh}", name=f"rb{hh}_{dd}_{sj}")
                r_ = rp.tile([1, SW], F32, tag=f"r{hh}", name=f"r{hh}_{dd}_{sj}")
                nc.vector.tensor_copy(rb_[0:1, :], pv[HD:HD + 1, :])
                nc.vector.reciprocal_approx_fast(out=r_[0:1, :], in_=rb_[0:1, :])
                nc.gpsimd.partition_broadcast(rb_[0:64, :], r_[0:1, :])
                if hh == 0:
                    nc.vector.tensor_mul(aocur[dd][0:64, :], pv[0:64, :], rb_[0:64, :])
                else:
                    # DVE cross-quadrant write: shift h1's normalized output up
                    # to partitions 64:128 without a DMA
                    nc.vector.tensor_mul(aocur[dd][64:128, :], pv[0:64, :], rb_[0:64, :])

            def emit_att(dd, sj, qt):
                last = 4 * sj + 3
                pv0 = pvp.tile([128, SW], F32, tag="pv0", name=f"pvh0_{dd}_{sj}")
                pv1 = pvp.tile([128, SW], F32, tag="pv1", name=f"pvh1_{dd}_{sj}")
                ao = aop.tile([128, SW], mdt, tag=f"aoT{dd}", name=f"aoT{dd}_{sj}")
                aocur[dd] = ao
                pending = emit_scores(dd, sj, 0, qt)
                for i in range(last + 1):
                    nxt = emit_scores(dd, sj, i + 1, qt) if i < last else None
                    # pop BEFORE the tail: PE is strict FIFO, so filler queued
                    # here executes during the exp latency that gates the PV
                    pop_filler(1)
                    emit_tail(dd, sj, i, pending, pv0, pv1, last)
                    pending = nxt
                emit_norm(dd, sj, pv0, 0)
                emit_norm(dd, sj, pv1, 1)
                return ao

            def emit_wo(sj, ltt, ao_tiles):
                # one token tile, both 512-col halves in one 2-bank psum tile
                tt = 4 * sj + ltt
                tok = slice(128 * ltt, 128 * (ltt + 1))
                ps = ps2.tile([128, 2 * SW], F32, tag="sc", name=f"o{tt}")
                for ee in range(2):
                    for dd in range(4):
                        nc.tensor.matmul(
                            ps[:, SW * ee:SW * (ee + 1)],
                            mc(ao_tiles[dd][:, tok]), mc(wot[:, dd, SW * ee:SW * (ee + 1)]),
                            start=(dd == 0), stop=(dd == 3))
                st = sp3.tile([128, 2 * SW], F32, tag="st", name=f"st{tt}")
                nc.vector.tensor_copy(st[:], ps[:])
                nc.sync.dma_start(out_d[128 * tt:128 * (tt + 1), :], st[:])

            def queue_wo(sj, ltt, ao_tiles):
                # same as emit_wo, but as filler closures: the last swath has
                # no projection filler, so spread wo into its exp bubbles
                tt = 4 * sj + ltt
                tok = slice(128 * ltt, 128 * (ltt + 1))
                box = {}

                def mmstep(ee, dd2, box=box):
                    if ee == 0 and dd2 == 0:
                        box["ps"] = ps2.tile([128, 2 * SW], F32, tag="sc",
                                             name=f"o{tt}")
                    nc.tensor.matmul(
                        box["ps"][:, SW * ee:SW * (ee + 1)],
                        mc(ao_tiles[dd2][:, tok]),
                        mc(wot[:, dd2, SW * ee:SW * (ee + 1)]),
                        start=(dd2 == 0), stop=(dd2 == 3))

                def fin(box=box):
                    st = sp3.tile([128, 2 * SW], F32, tag="st", name=f"st{tt}")
                    nc.vector.tensor_copy(st[:], box["ps"][:])
                    nc.sync.dma_start(out_d[128 * tt:128 * (tt + 1), :], st[:])

                for ee in range(2):
                    for dd2 in range(4):
                        filler.append(lambda ee=ee, dd2=dd2: mmstep(ee, dd2))
                filler.append(fin)

            # ---------------- weave ----------------
            # first x swath in two chunks woven with the rest of wqt, so the
            # opening projection steps start after ~1/4 of the prologue bytes
            nc.sync.dma_start(xs[0][:, 0:2], xT_r[:, 0:2, 0:SW])
            nc.sync.dma_start(wqt[:, 2:5], wqT_r[:, 2:5])
            nc.sync.dma_start(xs[0][:, 2:5], xT_r[:, 2:5, 0:SW])
            nc.sync.dma_start(wqt[:, 5:ND], wqT_r[:, 5:ND])
            nc.sync.dma_start(xs[0][:, 5:ND], xT_r[:, 5:ND, 0:SW])
            nc.sync.dma_start(bqT[:], bqT_d[:])
            nc.sync.dma_start(cmT[:], cmT_d[:])
            nc.sync.dma_start(id128[:], id_d[:])
            nc.sync.dma_start(wkt[:, 0:4], wkT_r[:, 0:4])
            nc.sync.dma_start(wkt[:, 4:ND], wkT_r[:, 4:ND])
            nc.sync.dma_start(xs[1][:], xT_r[:, :, SW:2 * SW])
            nc.sync.dma_start(wvt[:], wvT_r[:])
            nc.sync.dma_start(wot[:], woT_r[:])
            # flush only what swath-0 attention needs up front; q23/k23 stay
            # queued as filler for the first attention blocks
            proj_pair_qk(0, xs[0], "q", 0, 1)
            proj_pair_qk(0, xs[0], "k", 0, 1)
            proj_pair_v(0, xs[0], 0, 1)
            proj_pair_v(0, xs[0], 2, 3)
            pop_filler(len(filler))
            proj_pair_qk(0, xs[0], "q", 2, 3)
            proj_pair_qk(0, xs[0], "k", 2, 3)

            ao_prev = None
            for sj in range(NSW):
                if sj + 2 < NSW:
                    # persistent tiles: no WAR gate, DMA runs as soon as the
                    # queue reaches it
                    nc.sync.dma_start(xs[sj + 2][:],
                                      xT_r[:, :, SW * (sj + 2):SW * (sj + 3)])
                if sj + 1 < NSW:
                    # queue next swath's projections; they emit as filler
                    proj_pair_qk(sj + 1, xs[sj + 1], "q", 0, 1)
                    proj_pair_qk(sj + 1, xs[sj + 1], "q", 2, 3)
                    proj_pair_qk(sj + 1, xs[sj + 1], "k", 0, 1)
                    proj_pair_qk(sj + 1, xs[sj + 1], "k", 2, 3)
                    proj_pair_v(sj + 1, xs[sj + 1], 0, 1)
                    proj_pair_v(sj + 1, xs[sj + 1], 2, 3)
                ao_now = [None] * 4
                for dd in range(4):
                    if sj == NSW - 1 and ao_prev is not None:
                        # last swath has no proj filler; weave wo(sj-1) into
                        # the attention i-loop instead of bursting it after
                        queue_wo(sj - 1, dd, ao_prev)
                    ao_now[dd] = emit_att(dd, sj, qtab[sj][dd])
                    # drain some filler between blocks, plus wo for sj-1
                    pop_filler(3 if sj > 0 else 9)
                    if sj != NSW - 1 and ao_prev is not None:
                        emit_wo(sj - 1, dd, ao_prev)
                ao_prev = ao_now
                pop_filler(len(filler))  # flush: next swath's q/k/v must be ready
            # final swath's wo: emit each tile's dd=0..2 parts first (their ao
            # is ready during dd=3's attention), dd=3 part + copy last, so the
            # PE chews matmuls while the last norm chain completes on
            # vector/gpsimd. Stagger to 3 live chains (ps2 pool depth).
            wops = {}

            def wo_part(ltt, dds, start, stop):
                tok = slice(128 * ltt, 128 * (ltt + 1))
                if ltt not in wops:
                    wops[ltt] = ps2.tile([128, 2 * SW], F32, tag="sc",
                                         name=f"o{4 * (NSW - 1) + ltt}")
                ps = wops[ltt]
                for ee in range(2):
                    for dd2 in dds:
                        nc.tensor.matmul(
                            ps[:, SW * ee:SW * (ee + 1)],
                            mc(ao_prev[dd2][:, tok]),
                            mc(wot[:, dd2, SW * ee:SW * (ee + 1)]),
                            start=(start and dd2 == dds[0]),
                            stop=(stop and dd2 == dds[-1]))

            def wo_fin(ltt, half=None):
                # per-half copies pipeline against the remaining matmuls
                tt = 4 * (NSW - 1) + ltt
                if ltt not in wost:
                    wost[ltt] = sp3.tile([128, 2 * SW], F32, tag="st",
                                         name=f"st{tt}")
                st = wost[ltt]
                halves = range(2) if half is None else (half,)
                for ee in halves:
                    cols = slice(SW * ee, SW * (ee + 1))
                    nc.vector.tensor_copy(st[:, cols], wops[ltt][:, cols])
                if half is None or half == 1:
                    nc.sync.dma_start(out_d[128 * tt:128 * (tt + 1), :], st[:])

            def wo_last(ltt):
                # final dd=3 contribution per half, each half's copy issued
                # immediately so it overlaps the other half's matmul
                tok = slice(128 * ltt, 128 * (ltt + 1))
                for ee in range(2):
                    nc.tensor.matmul(
                        wops[ltt][:, SW * ee:SW * (ee + 1)],
                        mc(ao_prev[3][:, tok]),
                        mc(wot[:, 3, SW * ee:SW * (ee + 1)]),
                        start=False, stop=True)
                    wo_fin(ltt, half=ee)

            wost = {}
            for ltt in (0, 1, 2):
                wo_part(ltt, [0, 1, 2], start=True, stop=False)
            wo_last(0)
            wo_part(3, [0, 1, 2], start=True, stop=False)
            for ltt in (1, 2, 3):
                wo_last(ltt)

    nc.compile()
    return nc


def _get_nc(mode):
    if mode not in _NC_CACHE:
        _NC_CACHE[mode] = _build(mode)
    return _NC_CACHE[mode]


def _causal_mask_tiles():
    # additive triangle for a diagonal 128-block (keep iff q >= p), returned
    # TRANSPOSED for the PE-side mask preload (psum := cmT.T @ I = cm), plus
    # the identity used as the preload's moving operand
    p = np.arange(128)[:, None]
    q = np.arange(128)[None, :]
    cm = np.where(q >= p, np.float32(0.0), np.float32(-1e30)).astype(np.float32)
    return np.ascontiguousarray(cm.T), np.eye(128, dtype=np.float32)


def kernel(x, mask, wq, bq, wk, bk, wv, bv, wo, bo):
    x = np.asarray(x, dtype=np.float32)
    wq = np.asarray(wq, dtype=np.float32)
    bq = np.asarray(bq, dtype=np.float32)
    wk = np.asarray(wk, dtype=np.float32)
    wv = np.asarray(wv, dtype=np.float32)
    bv = np.asarray(bv, dtype=np.float32)
    wo = np.asarray(wo, dtype=np.float32)
    bo = np.asarray(bo, dtype=np.float32)
    # mask is the causal tril (hardcoded in the kernel); bk cancels in softmax

    nc = _get_nc(MODE)
    _, np_dt = _mm_dt(MODE)

    cmT, id128 = _causal_mask_tiles()
    in_maps = []
    for c in range(8):
        b, hg = c // 2, c % 2
        rows = slice(DPC * hg, DPC * (hg + 1))
        in_maps.append({
            "xT": np.ascontiguousarray(x[b].T).astype(np_dt),
            "wqT": np.ascontiguousarray(wq[rows].T).astype(np_dt),
            "wkT": np.ascontiguousarray(wk[rows].T).astype(np_dt),
            "wvT": np.ascontiguousarray(wv[rows].T).astype(np_dt),
            "woT": np.ascontiguousarray(wo[:, rows].T).astype(np_dt),
            "bqT": np.ascontiguousarray(bq[rows].reshape(4, 128).T).astype(np.float32),
            "cmT": cmT.astype(np_dt),
            "id128": id128.astype(np_dt),
        })

    res = run_bass_kernel_spmd(nc, in_maps, list(range(8))).results

    corr = (wo @ bv) + bo  # bv commutes through softmax-normalized attention
    out = np.empty((B, S, D), dtype=np.float32)
    for b in range(B):
        out[b] = res[2 * b]["out"] + res[2 * b + 1]["out"] + corr
    return out



# revision 67
# speedup vs baseline: 1.1848x; 1.0012x over previous
"""Multi-head causal attention (B=4, S=2048, D=1024, H=16, Hd=64) on 8 trn2 cores.

Sharding: data-parallel over batch (4) x tensor-parallel over heads (2 groups
of 8 heads). Core c handles batch c//2 and heads 8*(c%2)..8*(c%2)+7:
  - wq/wk/wv column-parallel (each core owns 512 of the 1024 output dims),
  - wo row-parallel (partial outputs summed on host).

Device-side per core:
  phase 1: qT/kT (transposed, [dq,S]) and v (natural, [S,hd]) projections
  phase 2: per head-pair d, q-swath j: scoresT = kT.T-chunk @ qT-swath (row-
           tiled pair of K=64 matmuls), causal additive mask on diagonal
           tiles, exp on ACT (no max subtraction: scores are O(1), exp is
           safe), PV matmul with a ones-column appended to v so the softmax
           denominator falls out of the same matmul, then normalize.
  phase 3: out_partial = attnT.T @ woT  (row-parallel wo)

Host side: shard/transposes, pair-sum of partials, + wo@bv + bo correction
(bk provably cancels in softmax; bv commutes to a constant because softmax
rows sum to 1).

Math note: softmax computed without max-subtraction (scores ~ N(0,1), exp
overflow impossible in fp32); masked entries get -1e30 pre-exp -> exp = 0.
"""
import sys

sys.path.insert(0, "/opt/trn_rl_repo")

import numpy as np

from concourse import bacc, mybir, tile
from concourse.bass_utils import run_bass_kernel_spmd

B, S, D = 4, 2048, 1024
H, HD = 16, 64
HPC = 8        # heads per core
DPC = HPC * HD  # 512 projection dims per core
SW = 512       # q swath width
NSW = S // SW  # 4
NT = S // 128  # 16 token tiles
ND = D // 128  # 8 contraction chunks

# matmul dtype mode: "f32" (exact, 4x slow), "f32r" (full speed, ~tf32ish),
# "bf16" (full speed, least precise, half DMA/SBUF)
MODE = "bf16"

F32 = mybir.dt.float32
EXPF = mybir.ActivationFunctionType.Exp

_NC_CACHE = {}


def _mm_dt(mode):
    import ml_dtypes
    if mode == "bf16":
        return mybir.dt.bfloat16, ml_dtypes.bfloat16
    if mode in ("f32r", "f32r_hi"):
        # float32r: fp32 storage, PE reads reduced mantissa at full rate.
        # np-side arrays stay fp32.
        return mybir.dt.float32r, np.float32
    return F32, np.float32


def _build(mode):
    mdt, _ = _mm_dt(mode)
    # PV-stage dtype: bf16 operands (exp weights + v) halve SBUF at a
    # ~2e-3 rel-err cost; f32r_hi keeps them f32r (~3.5e-4) with tighter
    # buffer counts to fit SBUF.
    pdt = mybir.dt.bfloat16 if mode == "f32r" else mdt
    hi = mode != "f32r"

    def mc(ap):
        return ap

    nc = bacc.Bacc("TRN2", target_bir_lowering=False, debug=False, num_devices=8)

    xT_d = nc.dram_tensor("xT", [D, S], mdt, kind="ExternalInput").ap()
    wqT_d = nc.dram_tensor("wqT", [D, DPC], mdt, kind="ExternalInput").ap()
    wkT_d = nc.dram_tensor("wkT", [D, DPC], mdt, kind="ExternalInput").ap()
    wvT_d = nc.dram_tensor("wvT", [D, DPC], mdt, kind="ExternalInput").ap()
    woT_d = nc.dram_tensor("woT", [DPC, D], mdt, kind="ExternalInput").ap()
    bqT_d = nc.dram_tensor("bqT", [128, 4], F32, kind="ExternalInput").ap()
    cmT_d = nc.dram_tensor("cmT", [128, 128], mdt, kind="ExternalInput").ap()
    id_d = nc.dram_tensor("id128", [128, 128], mdt, kind="ExternalInput").ap()
    out_d = nc.dram_tensor("out", [S, D], mdt, kind="ExternalOutput").ap()

    xT_r = xT_d.rearrange("(c p) s -> p c s", p=128)
    wqT_r = wqT_d.rearrange("(c p) n -> p c n", p=128)
    wkT_r = wkT_d.rearrange("(c p) n -> p c n", p=128)
    wvT_r = wvT_d.rearrange("(c p) n -> p c n", p=128)
    woT_r = woT_d.rearrange("(c p) n -> p c n", p=128)

    with tile.TileContext(nc) as tc:
        with (
            tc.tile_pool(name="pers", bufs=1) as pp,
            tc.tile_pool(name="qts", bufs=2) as qp,
            tc.tile_pool(name="aots", bufs=3) as aop,
            tc.tile_pool(name="xp", bufs=1 if hi else 3) as xp,
            tc.tile_pool(name="exp", bufs=3 if hi else 5) as ep,
            tc.tile_pool(name="rp", bufs=2) as rp,
            tc.tile_pool(name="stp", bufs=4) as sp3,
            tc.tile_pool(name="scp", bufs=3, space="PSUM") as ps2,
            tc.tile_pool(name="pvp", bufs=1, space="PSUM") as pvp,
        ):
            kT = [pp.tile([128, S], mdt, tag=f"kT{d}", name=f"kT{d}") for d in range(4)]
            v3 = [pp.tile([128, HPC, HD + 1], pdt, tag=f"v{t}", name=f"v{t}") for t in range(NT)]
            wqt = pp.tile([128, ND, DPC], mdt, tag="wqt", name="wqt")
            wkt = pp.tile([128, ND, DPC], mdt, tag="wkt", name="wkt")
            wvt = pp.tile([128, ND, DPC], mdt, tag="wvt", name="wvt")
            wot = pp.tile([128, 4, D], mdt, tag="wot", name="wot")
            bqT = pp.tile([128, 4], F32, tag="bqT", name="bqT")
            zb = pp.tile([128, 1], F32, tag="zb", name="zb")
            ones8 = pp.tile([128, HPC], F32, tag="ones8", name="ones8")
            cmT = pp.tile([128, 128], mdt, tag="cmT", name="cmT")
            id128 = pp.tile([128, 128], mdt, tag="id128", name="id128")
            # wqt first: the opening projections only need it (+ xsw(0));
            # chunked so the first proj steps start after ~1/4 of the bytes.
            # bqT/cmT/id128 are only consumed from the first drain/attention
            # on, so they queue behind the critical path.
            nc.sync.dma_start(wqt[:, 0:1], wqT_r[:, 0:1])
            nc.vector.memset(zb[:], 0.0)
            nc.vector.memset(ones8[:], 1.0)

            qtab = {sj: [None] * 4 for sj in range(NSW)}  # per (sj, dd) qT tile
            aocur = [None] * 4   # per-dd current swath attnT tile

            # persistent per-swath x tiles: unique tags mean no pool-rotation
            # WAR gates, so every x DMA can run during the prologue
            xs = [pp.tile([128, ND, SW], mdt, tag=f"xsw{j}", name=f"xsw{j}")
                  for j in range(NSW)]

            filler = []  # FIFO of emission closures (each ~2 matmuls of filler)

            def proj_pair_qk(sj, xsw, which, da, db):
                # two projection outputs share one 2-bank psum tile; dk-steps
                # are queued as filler closures woven into attention i-loops
                wt = wqt if which == "q" else wkt
                box = {}

                def step(dk, box=box):
                    if dk == 0:
                        box["ps"] = ps2.tile([128, 2 * SW], F32, tag="sc", name=f"p{which}{sj}_{da}")
                    ps = box["ps"]
                    nc.tensor.matmul(
                        ps[:, 0:SW], mc(wt[:, dk, 128 * da:128 * da + 128]),
                        mc(xsw[:, dk, :]), start=(dk == 0), stop=(dk == ND - 1))
                    nc.tensor.matmul(
                        ps[:, SW:2 * SW], mc(wt[:, dk, 128 * db:128 * db + 128]),
                        mc(xsw[:, dk, :]), start=(dk == 0), stop=(dk == ND - 1))

                def drain(box=box):
                    ps = box["ps"]
                    cols = slice(SW * sj, SW * (sj + 1))
                    if which == "q":
                        for half, dd in ((0, da), (1, db)):
                            qt = qp.tile([128, SW], mdt, tag=f"qT{dd}", name=f"qT{dd}_{sj}")
                            nc.vector.tensor_scalar_add(
                                qt[:], ps[:, SW * half:SW * (half + 1)], bqT[:, dd:dd + 1])
                            qtab[sj][dd] = qt
                    else:
                        for half, dd in ((0, da), (1, db)):
                            nc.vector.tensor_copy(kT[dd][:, cols], ps[:, SW * half:SW * (half + 1)])

                for dk in range(ND):
                    filler.append(lambda dk=dk: step(dk))
                filler.append(drain)

            def proj_pair_v(sj, xsw, ta, tb):
                box = {}

                def step(dk, box=box):
                    if dk == 0:
                        box["ps"] = ps2.tile([128, 2 * SW], F32, tag="sc", name=f"pv{sj}_{ta}")
                    ps = box["ps"]
                    nc.tensor.matmul(
                        ps[:, 0:SW], mc(xsw[:, dk, 128 * ta:128 * ta + 128]),
                        mc(wvt[:, dk, :]), start=(dk == 0), stop=(dk == ND - 1))
                    nc.tensor.matmul(
                        ps[:, SW:2 * SW], mc(xsw[:, dk, 128 * tb:128 * tb + 128]),
                        mc(wvt[:, dk, :]), start=(dk == 0), stop=(dk == ND - 1))

                def drain(box=box):
                    ps = box["ps"]
                    for half, tloc in ((0, ta), (1, tb)):
                        t = 4 * sj + tloc
                        nc.vector.tensor_copy(
                            v3[t][:, :, 0:HD],
                            ps[:, SW * half:SW * (half + 1)].rearrange("p (h e) -> p h e", h=HPC))
                        nc.vector.tensor_copy(v3[t][:, :, HD:HD + 1].squeeze(), ones8[:])

                for dk in range(ND):
                    filler.append(lambda dk=dk: step(dk))
                filler.append(drain)

            def pop_filler(n):
                for _ in range(n):
                    if not filler:
                        return
                    filler.pop(0)()

            def emit_scores(dd, sj, i, qt):
                krows = slice(128 * i, 128 * (i + 1))
                # diagonal key tiles: queries below c0 are fully masked, skip
                # their score columns
                c0 = 128 * (i - 4 * sj) if i >= 4 * sj else 0
                ps = ps2.tile([128, 2 * SW], F32, tag="sc", name=f"sc{dd}_{sj}_{i}")
                if c0 == 0 and i < 4 * sj:
                    # off-diagonal tile: plain full-width scores
                    nc.tensor.matmul(ps[:, 0:SW], mc(kT[dd][0:64, krows]),
                                     mc(qt[0:64, :]))
                    nc.tensor.matmul(ps[:, SW:2 * SW], mc(kT[dd][64:128, krows]),
                                     mc(qt[64:128, :]))
                    return ps
                # diagonal tile: preload the additive causal mask into the
                # 128-wide diagonal block via a PE matmul (cmT @ I), then
                # accumulate scores on top; exp can then read psum directly
                # with no vector mask-add in the chain.
                c1 = c0 + 128
                for g in range(2):
                    nc.tensor.matmul(ps[:, g * SW + c0:g * SW + c1], mc(cmT[:]),
                                     mc(id128[:]), start=True, stop=False)
                nc.tensor.matmul(ps[:, c0:c1], mc(kT[dd][0:64, krows]),
                                 mc(qt[0:64, c0:c1]), start=False, stop=True)
                nc.tensor.matmul(ps[:, SW + c0:SW + c1], mc(kT[dd][64:128, krows]),
                                 mc(qt[64:128, c0:c1]), start=False, stop=True)
                if c1 < SW:
                    nc.tensor.matmul(ps[:, c1:SW], mc(kT[dd][0:64, krows]),
                                     mc(qt[0:64, c1:SW]))
                    nc.tensor.matmul(ps[:, SW + c1:2 * SW], mc(kT[dd][64:128, krows]),
                                     mc(qt[64:128, c1:SW]))
                return ps

            def emit_tail(dd, sj, i, ps, pv0, pv1, last):
                h0, h1 = 2 * dd, 2 * dd + 1
                t = i - 4 * sj
                c0 = 128 * t if t >= 0 else 0
                ex = ep.tile([128, 2 * SW], pdt, tag="ex", name=f"ex{dd}_{sj}_{i}")
                if t >= 0:
                    pse = ps[:].rearrange("p (g q) -> p g q", g=2)[:, :, c0:SW]
                    exe = ex[:].rearrange("p (g q) -> p g q", g=2)[:, :, c0:SW]
                    nc.scalar.activation(exe, pse, EXPF, bias=zb[:], scale=0.125)
                else:
                    nc.scalar.activation(ex[:], ps[:], EXPF, bias=zb[:], scale=0.125)
                nc.tensor.matmul(
                    pv0[0:HD + 1, c0:SW], mc(v3[i][:, h0, :]), mc(ex[:, c0:SW]),
                    start=(i == 0), stop=(i == last))
                nc.tensor.matmul(
                    pv1[0:HD + 1, c0:SW], mc(v3[i][:, h1, :]), mc(ex[:, SW + c0:2 * SW]),
                    start=(i == 0), stop=(i == last))

            def emit_norm(dd, sj, pv, hh):
                rb_ = rp.tile([64, SW], F32, tag=f"rb{hh}", name=f"rb{hh}_{dd}_{sj}")
                r_ = rp.tile([1, SW], F32, tag=f"r{hh}", name=f"r{hh}_{dd}_{sj}")
                nc.vector.tensor_copy(rb_[0:1, :], pv[HD:HD + 1, :])
                nc.vector.reciprocal_approx_fast(out=r_[0:1, :], in_=rb_[0:1, :])
                nc.gpsimd.partition_broadcast(rb_[0:64, :], r_[0:1, :])
                if hh == 0:
                    nc.vector.tensor_mul(aocur[dd][0:64, :], pv[0:64, :], rb_[0:64, :])
                else:
                    # DVE cross-quadrant write: shift h1's normalized output up
                    # to partitions 64:128 without a DMA
                    nc.vector.tensor_mul(aocur[dd][64:128, :], pv[0:64, :], rb_[0:64, :])

            def emit_att(dd, sj, qt):
                last = 4 * sj + 3
                pv0 = pvp.tile([128, SW], F32, tag="pv0", name=f"pvh0_{dd}_{sj}")
                pv1 = pvp.tile([128, SW], F32, tag="pv1", name=f"pvh1_{dd}_{sj}")
                ao = aop.tile([128, SW], mdt, tag=f"aoT{dd}", name=f"aoT{dd}_{sj}")
                aocur[dd] = ao
                pending = emit_scores(dd, sj, 0, qt)
                for i in range(last + 1):
                    nxt = emit_scores(dd, sj, i + 1, qt) if i < last else None
                    # pop BEFORE the tail: PE is strict FIFO, so filler queued
                    # here executes during the exp latency that gates the PV
                    pop_filler(1)
                    emit_tail(dd, sj, i, pending, pv0, pv1, last)
                    pending = nxt
                emit_norm(dd, sj, pv0, 0)
                emit_norm(dd, sj, pv1, 1)
                return ao

            def emit_wo(sj, ltt, ao_tiles):
                # one token tile, both 512-col halves in one 2-bank psum tile
                tt = 4 * sj + ltt
                tok = slice(128 * ltt, 128 * (ltt + 1))
                ps = ps2.tile([128, 2 * SW], F32, tag="sc", name=f"o{tt}")
                for ee in range(2):
                    for dd in range(4):
                        nc.tensor.matmul(
                            ps[:, SW * ee:SW * (ee + 1)],
                            mc(ao_tiles[dd][:, tok]), mc(wot[:, dd, SW * ee:SW * (ee + 1)]),
                            start=(dd == 0), stop=(dd == 3))
                st = sp3.tile([128, 2 * SW], mdt, tag="st", name=f"st{tt}")
                nc.vector.tensor_copy(st[:], ps[:])
                nc.sync.dma_start(out_d[128 * tt:128 * (tt + 1), :], st[:])

            def queue_wo(sj, ltt, ao_tiles):
                # same as emit_wo, but as filler closures: the last swath has
                # no projection filler, so spread wo into its exp bubbles
                tt = 4 * sj + ltt
                tok = slice(128 * ltt, 128 * (ltt + 1))
                box = {}

                def mmstep(ee, dd2, box=box):
                    if ee == 0 and dd2 == 0:
                        box["ps"] = ps2.tile([128, 2 * SW], F32, tag="sc",
                                             name=f"o{tt}")
                    nc.tensor.matmul(
                        box["ps"][:, SW * ee:SW * (ee + 1)],
                        mc(ao_tiles[dd2][:, tok]),
                        mc(wot[:, dd2, SW * ee:SW * (ee + 1)]),
                        start=(dd2 == 0), stop=(dd2 == 3))

                def fin(box=box):
                    st = sp3.tile([128, 2 * SW], mdt, tag="st", name=f"st{tt}")
                    nc.vector.tensor_copy(st[:], box["ps"][:])
                    nc.sync.dma_start(out_d[128 * tt:128 * (tt + 1), :], st[:])

                for ee in range(2):
                    for dd2 in range(4):
                        filler.append(lambda ee=ee, dd2=dd2: mmstep(ee, dd2))
                filler.append(fin)

            # ---------------- weave ----------------
            # first x swath in two chunks woven with the rest of wqt, so the
            # opening projection steps start after ~1/4 of the prologue bytes
            nc.sync.dma_start(xs[0][:, 0:1], xT_r[:, 0:1, 0:SW])
            nc.sync.dma_start(wqt[:, 1:2], wqT_r[:, 1:2])
            nc.sync.dma_start(xs[0][:, 1:2], xT_r[:, 1:2, 0:SW])
            nc.sync.dma_start(wqt[:, 2:5], wqT_r[:, 2:5])
            nc.sync.dma_start(xs[0][:, 2:5], xT_r[:, 2:5, 0:SW])
            nc.sync.dma_start(wqt[:, 5:ND], wqT_r[:, 5:ND])
            nc.sync.dma_start(xs[0][:, 5:ND], xT_r[:, 5:ND, 0:SW])
            nc.sync.dma_start(bqT[:], bqT_d[:])
            nc.sync.dma_start(cmT[:], cmT_d[:])
            nc.sync.dma_start(id128[:], id_d[:])
            nc.sync.dma_start(wkt[:, 0:4], wkT_r[:, 0:4])
            nc.sync.dma_start(wkt[:, 4:ND], wkT_r[:, 4:ND])
            nc.sync.dma_start(xs[1][:], xT_r[:, :, SW:2 * SW])
            nc.sync.dma_start(wvt[:], wvT_r[:])
            nc.sync.dma_start(wot[:], woT_r[:])
            # flush only what swath-0 attention needs up front; q23/k23 stay
            # queued as filler for the first attention blocks
            proj_pair_qk(0, xs[0], "q", 0, 1)
            proj_pair_qk(0, xs[0], "k", 0, 1)
            proj_pair_v(0, xs[0], 0, 1)
            proj_pair_v(0, xs[0], 2, 3)
            pop_filler(len(filler))
            proj_pair_qk(0, xs[0], "q", 2, 3)
            proj_pair_qk(0, xs[0], "k", 2, 3)

            ao_prev = None
            for sj in range(NSW):
                if sj + 2 < NSW:
                    # persistent tiles: no WAR gate, DMA runs as soon as the
                    # queue reaches it
                    nc.sync.dma_start(xs[sj + 2][:],
                                      xT_r[:, :, SW * (sj + 2):SW * (sj + 3)])
                if sj + 1 < NSW:
                    # queue next swath's projections; they emit as filler
                    proj_pair_qk(sj + 1, xs[sj + 1], "q", 0, 1)
                    proj_pair_qk(sj + 1, xs[sj + 1], "q", 2, 3)
                    proj_pair_qk(sj + 1, xs[sj + 1], "k", 0, 1)
                    proj_pair_qk(sj + 1, xs[sj + 1], "k", 2, 3)
                    proj_pair_v(sj + 1, xs[sj + 1], 0, 1)
                    proj_pair_v(sj + 1, xs[sj + 1], 2, 3)
                ao_now = [None] * 4
                for dd in range(4):
                    if sj == NSW - 1 and ao_prev is not None:
                        # last swath has no proj filler; weave wo(sj-1) into
                        # the attention i-loop instead of bursting it after
                        queue_wo(sj - 1, dd, ao_prev)
                    ao_now[dd] = emit_att(dd, sj, qtab[sj][dd])
                    # drain some filler between blocks, plus wo for sj-1
                    pop_filler(3 if sj > 0 else 9)
                    if sj != NSW - 1 and ao_prev is not None:
                        emit_wo(sj - 1, dd, ao_prev)
                ao_prev = ao_now
                pop_filler(len(filler))  # flush: next swath's q/k/v must be ready
            # final swath's wo: emit each tile's dd=0..2 parts first (their ao
            # is ready during dd=3's attention), dd=3 part + copy last, so the
            # PE chews matmuls while the last norm chain completes on
            # vector/gpsimd. Stagger to 3 live chains (ps2 pool depth).
            wops = {}

            def wo_part(ltt, dds, start, stop):
                tok = slice(128 * ltt, 128 * (ltt + 1))
                if ltt not in wops:
                    wops[ltt] = ps2.tile([128, 2 * SW], F32, tag="sc",
                                         name=f"o{4 * (NSW - 1) + ltt}")
                ps = wops[ltt]
                for ee in range(2):
                    for dd2 in dds:
                        nc.tensor.matmul(
                            ps[:, SW * ee:SW * (ee + 1)],
                            mc(ao_prev[dd2][:, tok]),
                            mc(wot[:, dd2, SW * ee:SW * (ee + 1)]),
                            start=(start and dd2 == dds[0]),
                            stop=(stop and dd2 == dds[-1]))

            def wo_fin(ltt, half=None):
                # per-half copies pipeline against the remaining matmuls
                tt = 4 * (NSW - 1) + ltt
                if ltt not in wost:
                    wost[ltt] = sp3.tile([128, 2 * SW], mdt, tag="st",
                                         name=f"st{tt}")
                st = wost[ltt]
                halves = range(2) if half is None else (half,)
                for ee in halves:
                    cols = slice(SW * ee, SW * (ee + 1))
                    nc.vector.tensor_copy(st[:, cols], wops[ltt][:, cols])
                if half is None or half == 1:
                    nc.sync.dma_start(out_d[128 * tt:128 * (tt + 1), :], st[:])

            def wo_last(ltt):
                # final dd=3 contribution per half, each half's copy issued
                # immediately so it overlaps the other half's matmul
                tok = slice(128 * ltt, 128 * (ltt + 1))
                for ee in range(2):
                    nc.tensor.matmul(
                        wops[ltt][:, SW * ee:SW * (ee + 1)],
                        mc(ao_prev[3][:, tok]),
                        mc(wot[:, 3, SW * ee:SW * (ee + 1)]),
                        start=False, stop=True)
                    wo_fin(ltt, half=ee)

            wost = {}
            for ltt in (0, 1, 2):
                wo_part(ltt, [0, 1, 2], start=True, stop=False)
            wo_last(0)
            wo_part(3, [0, 1, 2], start=True, stop=False)
            for ltt in (1, 2, 3):
                wo_last(ltt)

    nc.compile()
    return nc


def _get_nc(mode):
    if mode not in _NC_CACHE:
        _NC_CACHE[mode] = _build(mode)
    return _NC_CACHE[mode]


def _causal_mask_tiles():
    # additive triangle for a diagonal 128-block (keep iff q >= p), returned
    # TRANSPOSED for the PE-side mask preload (psum := cmT.T @ I = cm), plus
    # the identity used as the preload's moving operand
    p = np.arange(128)[:, None]
    q = np.arange(128)[None, :]
    cm = np.where(q >= p, np.float32(0.0), np.float32(-1e30)).astype(np.float32)
    return np.ascontiguousarray(cm.T), np.eye(128, dtype=np.float32)


def kernel(x, mask, wq, bq, wk, bk, wv, bv, wo, bo):
    x = np.asarray(x, dtype=np.float32)
    wq = np.asarray(wq, dtype=np.float32)
    bq = np.asarray(bq, dtype=np.float32)
    wk = np.asarray(wk, dtype=np.float32)
    wv = np.asarray(wv, dtype=np.float32)
    bv = np.asarray(bv, dtype=np.float32)
    wo = np.asarray(wo, dtype=np.float32)
    bo = np.asarray(bo, dtype=np.float32)
    # mask is the causal tril (hardcoded in the kernel); bk cancels in softmax

    nc = _get_nc(MODE)
    _, np_dt = _mm_dt(MODE)

    cmT, id128 = _causal_mask_tiles()
    in_maps = []
    for c in range(8):
        b, hg = c // 2, c % 2
        rows = slice(DPC * hg, DPC * (hg + 1))
        in_maps.append({
            "xT": np.ascontiguousarray(x[b].T).astype(np_dt),
            "wqT": np.ascontiguousarray(wq[rows].T).astype(np_dt),
            "wkT": np.ascontiguousarray(wk[rows].T).astype(np_dt),
            "wvT": np.ascontiguousarray(wv[rows].T).astype(np_dt),
            "woT": np.ascontiguousarray(wo[:, rows].T).astype(np_dt),
            "bqT": np.ascontiguousarray(bq[rows].reshape(4, 128).T).astype(np.float32),
            "cmT": cmT.astype(np_dt),
            "id128": id128.astype(np_dt),
        })

    res = run_bass_kernel_spmd(nc, in_maps, list(range(8))).results

    corr = (wo @ bv) + bo  # bv commutes through softmax-normalized attention
    out = np.empty((B, S, D), dtype=np.float32)
    for b in range(B):
        out[b] = (np.asarray(res[2 * b]["out"], dtype=np.float32)
                  + np.asarray(res[2 * b + 1]["out"], dtype=np.float32) + corr)
    return out

